# revision 1
# baseline (speedup 1.0000x reference)
"""MASA agent-attention kernel for Trainium2, 8-core SPMD.

Sharding: core = (batch b in 0..3) x (head-group hg in 0..1).
Each core computes conv1x1 + depthwise3x3 for its 4 heads' q/k/v/a
channels (384 of 768), the agent attention for those heads, and SimAM
over its 96 output channels. No cross-core communication.

Per-core channel order: [q(96), k(96), v(96), a(96)], head-major.
SBUF slabs of 128: s0 = q[0:96]+k[0:32], s1 = k[32:96]+v[0:64],
s2 = v[64:96]+a[0:96].

Engine-op partition windows must be 32-aligned and (base==0 or count<=32).
"""

import sys
import types
import numpy as np

import concourse.bacc as bacc
import concourse.bass as bass
import concourse.mybir as mybir
from concourse.tile import TileContext
from concourse.bass_utils import run_bass_kernel_spmd

F16 = mybir.dt.float16
F32 = mybir.dt.float32
AX = mybir.AxisListType
OP = mybir.AluOpType
AF = mybir.ActivationFunctionType

B, C, H, W = 4, 192, 128, 128
N = H * W              # 16384
M_AG = 64              # agent tokens
E_LAMBDA = 1e-4
RS = 130               # padded row stride for pre
PREFREE = RS * RS      # 16900

TAPS = [(dy, dx) for dy in (-1, 0, 1) for dx in (-1, 0, 1)]
# tap offset in pre: (1+dy)*RS + (1+dx); odd offsets (dx==0) are
# 4B-misaligned for fp16 2x mode -> always on PE. Extra PE taps for balance.
PE_TAPS = {
    0: TAPS,                                  # slab0 fully on PE
    1: [t for t in TAPS if t[1] == 0],        # center column
    2: [t for t in TAPS if t[1] == 0],
}
DVE_TAPS = {s: [t for t in TAPS if t not in PE_TAPS[s]] for s in range(3)}
WDIAG_SLOT = {}
for _s in range(3):
    for _t in PE_TAPS[_s]:
        WDIAG_SLOT[(_s, _t[0], _t[1])] = len(WDIAG_SLOT)
NDIAG = len(WDIAG_SLOT)

NB2 = 16               # block count for norm / attention / simam phases
BLK2 = 1024
NCH = 128              # s-chunks of 128 for k-side


def _install_ntff_hook():
    try:
        import antenv.axon_hooks  # noqa: F401
        return
    except ImportError:
        pass
    try:
        from trn_agent_boot.trn_boot import _ntff_profile_via_ctypes
        hook = _ntff_profile_via_ctypes('/opt/axon/libaxon_pjrt.so')
        mod = types.ModuleType("antenv.axon_hooks")
        mod.get_axon_ntff_profile_hook = lambda: hook
        mod.set_axon_ntff_profile_hook = lambda h: None
        sys.modules["antenv.axon_hooks"] = mod
    except Exception:
        pass


def build_nc(debug=False):
    nc = bacc.Bacc("TRN2", target_bir_lowering=False, debug=False, num_devices=8)

    # ---- DRAM I/O ----
    xin = nc.dram_tensor("xin", [192, N], F16, kind="ExternalInput").ap()
    w1a = nc.dram_tensor("w1a", [96, 384], F16, kind="ExternalInput").ap()
    w1b = nc.dram_tensor("w1b", [96, 384], F16, kind="ExternalInput").ap()
    wdiag = nc.dram_tensor("wdiag", [128, NDIAG * 128], F16, kind="ExternalInput").ap()
    wtap = nc.dram_tensor("wtap", [128, 27], F32, kind="ExternalInput").ap()
    tmp0 = nc.dram_tensor("tmp0", [48, 1], F32, kind="ExternalInput").ap()
    tmp1 = nc.dram_tensor("tmp1", [48, 1], F32, kind="ExternalInput").ap()
    pat = nc.dram_tensor("pat", [128, 240], F16, kind="ExternalInput").ap()
    out_d = nc.dram_tensor("out", [96, N], F32, kind="ExternalOutput").ap()
    if debug:
        dbg_pre = nc.dram_tensor("dbg_pre", [128, PREFREE], F16, kind="ExternalOutput").ap()
        dbg_q = nc.dram_tensor("dbg_q", [128, N], F16, kind="ExternalOutput").ap()
        dbg_k = nc.dram_tensor("dbg_k", [128, N], F16, kind="ExternalOutput").ap()
        dbg_qn = nc.dram_tensor("dbg_qn", [128, N], F16, kind="ExternalOutput").ap()
        dbg_ag = nc.dram_tensor("dbg_ag", [96, 256], F16, kind="ExternalOutput").ap()
        dbg_av0 = nc.dram_tensor("dbg_av0", [128, 48], F16, kind="ExternalOutput").ap()
        dbg_av1 = nc.dram_tensor("dbg_av1", [128, 48], F16, kind="ExternalOutput").ap()
        dbg_xa = nc.dram_tensor("dbg_xa", [96, N], F16, kind="ExternalOutput").ap()
        dbg_vt = nc.dram_tensor("dbg_vt", [128, 98 * 4], F16, kind="ExternalOutput").ap()
        dbg_avi = nc.dram_tensor("dbg_avi", [128, 48], F16, kind="ExternalOutput").ap()
        dbg_e1 = nc.dram_tensor("dbg_e1", [128, BLK2], F16, kind="ExternalOutput").ap()
        dbg_op = nc.dram_tensor("dbg_op", [128, BLK2], F32, kind="ExternalOutput").ap()
        dbg_rqs = nc.dram_tensor("dbg_rqs", [48, BLK2], F32, kind="ExternalOutput").ap()

    # ---- persistent SBUF ----
    scratch = nc.alloc_sbuf_tensor("scratch", [128, PREFREE], F16).ap()
    dw0 = nc.alloc_sbuf_tensor("dw0", [128, N], F16).ap()
    dw1 = nc.alloc_sbuf_tensor("dw1", [128, N], F16).ap()
    dw2 = nc.alloc_sbuf_tensor("dw2", [128, N], F16).ap()
    dws = [dw0, dw1, dw2]
    w1a_s = nc.alloc_sbuf_tensor("w1a_s", [96, 384], F16).ap()
    w1b_s = nc.alloc_sbuf_tensor("w1b_s", [96, 384], F16).ap()
    wdiag_s = nc.alloc_sbuf_tensor("wdiag_s", [128, NDIAG * 128], F16).ap()
    wtap_s = nc.alloc_sbuf_tensor("wtap_s", [128, 27], F32).ap()
    ones_q = nc.alloc_sbuf_tensor("ones_q", [96, 96], F16).ap()
    ones_kA = nc.alloc_sbuf_tensor("ones_kA", [32, 96], F16).ap()
    ones_kB = nc.alloc_sbuf_tensor("ones_kB", [64, 96], F16).ap()
    ag_full = nc.alloc_sbuf_tensor("ag_full", [96, 256], F16).ap()
    agf = nc.alloc_sbuf_tensor("agf", [96, M_AG], F32).ap()
    agfs = nc.alloc_sbuf_tensor("agfs", [96, M_AG], F16).ap()
    temp_rep = nc.alloc_sbuf_tensor("temp_rep", [96, 1], F32).ap()
    av_l0 = nc.alloc_sbuf_tensor("av_l0", [128, 48], F16).ap()
    av_l1 = nc.alloc_sbuf_tensor("av_l1", [128, 48], F16).ap()
    dv_ones = nc.alloc_sbuf_tensor("dv_ones", [128, 48], F16).ap()
    asum = nc.alloc_sbuf_tensor("asum", [128, M_AG], F32).ap()      # rows 32:128
    rq2a = nc.alloc_sbuf_tensor("rq2a", [128, 1], F32).ap()
    rq2b = nc.alloc_sbuf_tensor("rq2b", [128, 1], F32).ap()
    mu_parts = nc.alloc_sbuf_tensor("mu_parts", [48, 2 * NB2], F32).ap()
    d2_parts = nc.alloc_sbuf_tensor("d2_parts", [96, NB2], F32).ap()
    mu_neg = nc.alloc_sbuf_tensor("mu_neg", [96, 1], F32).ap()
    mub = nc.alloc_sbuf_tensor("mub", [48, 2], F32).ap()
    sden = nc.alloc_sbuf_tensor("sden", [96, 1], F32).ap()
    s_ch = nc.alloc_sbuf_tensor("s_ch", [96, 1], F32).ap()
    half_s = nc.alloc_sbuf_tensor("half_s", [96, 1], F32).ap()

    # aliases (sequential reuse of big buffers)
    pre3 = scratch.rearrange("p (y x) -> p y x", x=RS)   # padded conv out
    vT = scratch[:, 0:NCH * 98]                          # after dwconv
    sig = scratch[:, 0:N]                                # final sigmoid (rows 0:96)
    as1 = scratch[:, 13000:13000 + 2048].bitcast(F32)    # [128,1024] pool stage1
    x_attn = dw1[0:96, :]                                # [96, N] f16 (phase D)
    kfull = dw2[0:96, :]                                 # k-hat packed (phase B)
    d2 = dw2[0:96, :]                                    # (phase E, after B)

    with TileContext(nc) as tc:
        with (
            tc.tile_pool(name="xio", bufs=4) as xio,
            tc.tile_pool(name="pout", bufs=2) as pout,
            tc.tile_pool(name="work", bufs=2) as work,
            tc.tile_pool(name="work1", bufs=1) as work1,
            tc.tile_pool(name="ppsum", bufs=2, space="PSUM") as ppsum,
        ):
            # ================= init =================
            nc.sync.dma_start(out=w1a_s[:], in_=w1a[:])
            nc.sync.dma_start(out=w1b_s[:], in_=w1b[:])
            nc.sync.dma_start(out=wdiag_s[:], in_=wdiag[:])
            nc.sync.dma_start(out=wtap_s[:], in_=wtap[:])
            # static patterns
            nc.sync.dma_start(out=ones_q[:], in_=pat[0:96, 0:96])
            nc.sync.dma_start(out=ones_kA[:], in_=pat[0:32, 96:192])
            nc.sync.dma_start(out=ones_kB[:], in_=pat[32:96, 96:192])
            nc.gpsimd.memset(av_l0[:], 0.0)
            nc.gpsimd.memset(av_l1[:], 0.0)
            # D1-rep ones lhsT: col j<24 -> even head (rows 0:64),
            # j>=24 -> odd head (rows 64:128)
            nc.sync.dma_start(out=dv_ones[:, 0:24], in_=pat[:, 192:216])
            nc.sync.dma_start(out=dv_ones[:, 24:48], in_=pat[:, 216:240])
            nc.gpsimd.memset(ag_full[:], 0.0)
            nc.sync.dma_start(out=temp_rep[0:48, :], in_=tmp0[:])
            nc.sync.dma_start(out=temp_rep[48:96, :], in_=tmp1[:])
            nc.gpsimd.memset(half_s[:], 0.5)
            # pre borders (rows 0 and 129, cols 0 and 129)
            nc.gpsimd.memset(pre3[:, 0, :], 0.0)
            nc.gpsimd.memset(pre3[:, 129, :], 0.0)
            nc.gpsimd.memset(pre3[:, :, 0], 0.0)
            nc.gpsimd.memset(pre3[:, :, 129], 0.0)

            if debug:
                nc.sync.dma_start(out=dbg_avi[:], in_=dv_ones[:])
            # ================= sweep1: conv1x1 + dwconv ====
            for s in range(3):
                wa = w1a_s[:, s * 128:(s + 1) * 128]
                wb = w1b_s[:, s * 128:(s + 1) * 128]
                nblk = N // 1024  # 16 blocks of 1024 (8 y-rows)

                def conv_blk(j, s=s, wa=wa, wb=wb):
                    x0 = xio.tile([96, 1024], F16, tag="x")
                    x1 = xio.tile([96, 1024], F16, tag="x")
                    nc.sync.dma_start(out=x0[:], in_=xin[0:96, j * 1024:(j + 1) * 1024])
                    nc.sync.dma_start(out=x1[:], in_=xin[96:192, j * 1024:(j + 1) * 1024])
                    ps = ppsum.tile([128, 1024], F32, tag="pA")
                    for q in range(2):
                        sl = slice(q * 512, (q + 1) * 512)
                        nc.tensor.matmul(ps[:, sl], wa, x0[:, sl], start=True, stop=False)
                        nc.tensor.matmul(ps[:, sl], wb, x1[:, sl], start=False, stop=True)
                    nc.scalar.copy(pre3[:, 1 + 8 * j: 9 + 8 * j, 1:129], ps[:])

                def dw_blk(j, s=s):
                    dst = dws[s][:, j * 1024:(j + 1) * 1024]
                    pe_t = PE_TAPS[s]
                    dv_t = DVE_TAPS[s]
                    pd = None
                    if pe_t:
                        pd = ppsum.tile([128, 1024], F32, tag="pB")
                        for q in range(2):
                            for ti, (dy, dx) in enumerate(pe_t):
                                dg = wdiag_s[:, WDIAG_SLOT[(s, dy, dx)] * 128:
                                             (WDIAG_SLOT[(s, dy, dx)] + 1) * 128]
                                rv = pre3[:, 1 + dy + 8 * j + 4 * q: 5 + dy + 8 * j + 4 * q,
                                          1 + dx: 129 + dx]
                                nc.tensor.matmul(pd[:, q * 512:(q + 1) * 512], dg, rv,
                                                 start=(ti == 0), stop=(ti == len(pe_t) - 1))
                    first = True
                    for (dy, dx) in dv_t:
                        ti = s * 9 + TAPS.index((dy, dx))
                        w_sc = wtap_s[:, ti:ti + 1]
                        rv = pre3[:, 1 + dy + 8 * j: 9 + dy + 8 * j, 1 + dx: 129 + dx]
                        if first and pd is not None:
                            nc.vector.scalar_tensor_tensor(
                                out=dst, in0=rv, scalar=w_sc, in1=pd[:],
                                op0=OP.mult, op1=OP.add)
                        elif first:
                            nc.vector.tensor_scalar(
                                out=dst, in0=rv, scalar1=w_sc, scalar2=None, op0=OP.mult)
                        else:
                            nc.vector.scalar_tensor_tensor(
                                out=dst, in0=rv, scalar=w_sc, in1=dst,
                                op0=OP.mult, op1=OP.add)
                        first = False
                    if not dv_t:
                        nc.scalar.copy(dst, pd[:])

                conv_blk(0)
                for j in range(1, nblk):
                    conv_blk(j)
                    dw_blk(j - 1)
                dw_blk(nblk - 1)

            if debug:
                nc.sync.dma_start(out=dbg_pre[:], in_=scratch[:])
                nc.sync.dma_start(out=dbg_q[:], in_=dw0[:])
                nc.sync.dma_start(out=dbg_k[:], in_=dw1[:])
            # ================= agent pooling (a = dw2[32:128]) ======
            for w0 in (32, 64, 96):
                a3 = dw2[w0:w0 + 32, :].rearrange("p (a xi) -> p a xi", xi=16)
                nc.vector.reduce_sum(as1[w0:w0 + 32, :], a3, axis=AX.X)
                as3 = as1[w0:w0 + 32, :].rearrange("p (yb yi xb) -> p yb xb yi",
                                                   yb=8, yi=16, xb=8)
                asum3 = asum[w0:w0 + 32, :].rearrange("p (yb xb) -> p yb xb", yb=8)
                nc.vector.reduce_sum(asum3, as3, axis=AX.X)
            nc.sync.dma_start(out=agf[0:48, :], in_=asum[32:80, :])
            nc.sync.dma_start(out=agf[48:96, :], in_=asum[80:128, :])
            # scale by temp/256 (per-partition scalar), then place blocks by DMA
            nc.vector.tensor_scalar(out=agfs[:], in0=agf[:],
                                    scalar1=temp_rep[:], scalar2=1.0 / 256.0,
                                    op0=OP.mult, op1=OP.mult)
            for h in range(4):
                nc.sync.dma_start(
                    out=ag_full[h * 24:(h + 1) * 24, h * 64:(h + 1) * 64],
                    in_=agfs[h * 24:(h + 1) * 24, :])

            # ================= l2norm of q, k =========================
            for j in range(NB2):
                blk = slice(j * BLK2, (j + 1) * BLK2)
                sq0 = work1.tile([128, BLK2], F16, tag="sq0")
                sq1 = work1.tile([64, BLK2], F16, tag="sq1")
                sqk = work1.tile([32, BLK2], F16, tag="sqk")
                nc.gpsimd.tensor_tensor(out=sq0[:], in0=dw0[:, blk], in1=dw0[:, blk],
                                        op=OP.mult)
                nc.gpsimd.tensor_tensor(out=sq1[:], in0=dw1[0:64, blk],
                                        in1=dw1[0:64, blk], op=OP.mult)
                nc.sync.dma_start(out=sqk[:], in_=sq0[96:128, :])
                pq = ppsum.tile([96, BLK2], F32, tag="pA")
                pk = ppsum.tile([96, BLK2], F32, tag="pB")
                for q in range(2):
                    sl = slice(q * 512, (q + 1) * 512)
                    nc.tensor.matmul(pq[:, sl], ones_q[:], sq0[0:96, sl],
                                     start=True, stop=True)
                    nc.tensor.matmul(pk[:, sl], ones_kA[:], sqk[:, sl],
                                     start=True, stop=False)
                    nc.tensor.matmul(pk[:, sl], ones_kB[:], sq1[:, sl],
                                     start=False, stop=True)
                rns_q = work1.tile([96, BLK2], F32, tag="rns_q")
                rns_k = work1.tile([96, BLK2], F32, tag="rns_k")
                nc.vector.reciprocal_approx_fast(out=rns_q[:], in_=pq[:])
                nc.vector.reciprocal_approx_fast(out=rns_k[:], in_=pk[:])
                rinv_q = work1.tile([96, BLK2], F16, tag="rinv_q")
                rinv_k = work1.tile([96, BLK2], F16, tag="rinv_k")
                nc.scalar.activation(rinv_q[:], rns_q[:], AF.Sqrt)
                nc.scalar.activation(rinv_k[:], rns_k[:], AF.Sqrt)
                rrk = work1.tile([128, BLK2], F16, tag="rrk")
                nc.sync.dma_start(out=rrk[96:128, :], in_=rinv_k[0:32, :])
                nc.sync.dma_start(out=rrk[0:64, :], in_=rinv_k[32:96, :])
                nc.gpsimd.tensor_tensor(out=dw0[0:96, blk], in0=dw0[0:96, blk],
                                        in1=rinv_q[:], op=OP.mult)
                nc.gpsimd.tensor_tensor(out=dw0[96:128, blk], in0=dw0[96:128, blk],
                                        in1=rrk[96:128, :], op=OP.mult)
                nc.gpsimd.tensor_tensor(out=dw1[0:64, blk], in0=dw1[0:64, blk],
                                        in1=rrk[0:64, :], op=OP.mult)

            if debug:
                nc.sync.dma_start(out=dbg_qn[:], in_=dw0[:])
            # ================= vT build (into scratch) ===================
            # per-chunk layout [one | v-ch 0:96 | one]: cols 0, 1:97, 97
            vT3 = vT.rearrange("p (c w) -> p c w", w=98)
            nc.gpsimd.memset(vT3[:, :, 0], 1.0)
            nc.gpsimd.memset(vT3[:, :, 97], 1.0)
            for c in range(NCH):
                ssl = slice(c * 128, (c + 1) * 128)
                for pi, vsrc in enumerate((dw1[64:96, ssl], dw1[96:128, ssl],
                                           dw2[0:32, ssl])):
                    st = work.tile([128, 32], F16, tag="vst")
                    nc.sync.dma_start(out=st[:], in_=vsrc, transpose=True)
                    nc.sync.dma_start(out=vT3[:, c, 1 + 32 * pi:33 + 32 * pi],
                                      in_=st[:])

            # ====== pack k-hat contiguous at base 0 (into dw2 rows 0:96) ==
            # (v rows of dw2 already consumed by vT; a rows by pooling)
            nc.sync.dma_start(out=kfull[0:32, :], in_=dw0[96:128, :])
            nc.sync.dma_start(out=kfull[32:96, :], in_=dw1[0:64, :])

            # ================= k-side: L2T -> exp -> agent_v =============
            agv0 = ppsum.tile([128, 49], F32, tag="pB")
            agv1 = ppsum.tile([128, 49], F32, tag="pB")
            for c in range(NCH):
                ssl = slice(c * 128, (c + 1) * 128)
                l2 = ppsum.tile([128, 256], F32, tag="pA")
                nc.tensor.matmul(l2[:], kfull[:, ssl], ag_full[:],
                                 start=True, stop=True)
                e2t = work.tile([128, 256], F16, tag="e2t")
                nc.scalar.activation(e2t[:], l2[:], AF.Exp)
                nc.tensor.matmul(agv0[:], e2t[:, 0:128], vT3[:, c, 0:49],
                                 start=(c == 0), stop=(c == NCH - 1))
                nc.tensor.matmul(agv1[:], e2t[:, 128:256], vT3[:, c, 49:98],
                                 start=(c == 0), stop=(c == NCH - 1))
            # agv0: D2 col 0, channels cols 1:49. agv1: channels 0:48, D2 col 48.
            nc.vector.reciprocal_approx_fast(out=rq2a[:], in_=agv0[:, 0:1])
            nc.vector.reciprocal_approx_fast(out=rq2b[:], in_=agv1[:, 48:49])
            # block-diagonal: even head of pair -> rows 0:64 x cols 0:24,
            # odd head -> rows 64:128 x cols 24:48 (other entries stay zero)
            nc.vector.tensor_scalar(out=av_l0[0:64, 0:24], in0=agv0[0:64, 1:25],
                                    scalar1=rq2a[0:64, :], scalar2=None, op0=OP.mult)
            for w0 in (64, 96):
                nc.vector.tensor_scalar(out=av_l0[w0:w0 + 32, 24:48],
                                        in0=agv0[w0:w0 + 32, 25:49],
                                        scalar1=rq2a[w0:w0 + 32, :], scalar2=None,
                                        op0=OP.mult)
            nc.vector.tensor_scalar(out=av_l1[0:64, 0:24], in0=agv1[0:64, 0:24],
                                    scalar1=rq2b[0:64, :], scalar2=None, op0=OP.mult)
            for w0 in (64, 96):
                nc.vector.tensor_scalar(out=av_l1[w0:w0 + 32, 24:48],
                                        in0=agv1[w0:w0 + 32, 24:48],
                                        scalar1=rq2b[w0:w0 + 32, :], scalar2=None,
                                        op0=OP.mult)

            if debug:
                nc.sync.dma_start(out=dbg_ag[:], in_=ag_full[:])
                nc.sync.dma_start(out=dbg_av0[:], in_=av_l0[:])
                nc.sync.dma_start(out=dbg_av1[:], in_=av_l1[:])
                nc.sync.dma_start(out=dbg_vt[:], in_=vT[:, 0:98 * 4])
            # ================= q-side + division =========================
            # out' psum rows: 0:48 channels, 64:88 D1-rep (even head of pair),
            # 96:120 D1-rep (odd head of pair)
            for hp in range(2):
                av_l = av_l0 if hp == 0 else av_l1
                ag_cols = ag_full[:, hp * 128:(hp + 1) * 128]
                for j in range(NB2):
                    blk = slice(j * BLK2, (j + 1) * BLK2)
                    l1 = ppsum.tile([128, BLK2], F32, tag="pA")
                    for q in range(2):
                        sl = slice(j * BLK2 + q * 512, j * BLK2 + (q + 1) * 512)
                        psl = slice(q * 512, (q + 1) * 512)
                        nc.tensor.matmul(l1[:, psl], ag_cols, dw0[0:96, sl],
                                         start=True, stop=True)
                    e1 = work.tile([128, BLK2], F16, tag="e1")
                    nc.scalar.activation(e1[:], l1[:], AF.Exp)
                    op_ = ppsum.tile([48, BLK2], F32, tag="pB")
                    od_ = ppsum.tile([48, BLK2], F32, tag="pB")
                    for q in range(2):
                        psl = slice(q * 512, (q + 1) * 512)
                        nc.tensor.matmul(op_[:, psl], av_l[:], e1[:, psl],
                                         start=True, stop=True)
                        nc.tensor.matmul(od_[:, psl], dv_ones[:], e1[:, psl],
                                         start=True, stop=True)
                    rqs = work1.tile([48, BLK2], F32, tag="rqs")
                    nc.vector.reciprocal_approx_fast(out=rqs[:], in_=od_[:])
                    if debug and hp == 0 and j == 0:
                        nc.sync.dma_start(out=dbg_e1[:], in_=e1[:])
                        opc = work1.tile([48, BLK2], F32, tag="opc")
                        nc.scalar.copy(opc[:], op_[:])
                        nc.sync.dma_start(out=dbg_op[0:48, :], in_=opc[:])
                        nc.sync.dma_start(out=dbg_rqs[:], in_=rqs[:])
                    if hp == 0:
                        nc.vector.scalar_tensor_tensor(
                            out=x_attn[0:48, blk], in0=op_[:], scalar=0.0,
                            in1=rqs[:], op0=OP.bypass, op1=OP.mult,
                            accum_out=mu_parts[:, j:j + 1])
                    else:
                        xt = work1.tile([48, BLK2], F16, tag="xt")
                        nc.vector.scalar_tensor_tensor(
                            out=xt[:], in0=op_[:], scalar=0.0,
                            in1=rqs[:], op0=OP.bypass, op1=OP.mult,
                            accum_out=mu_parts[:, NB2 + j:NB2 + j + 1])
                        nc.sync.dma_start(out=dw1[48:96, blk], in_=xt[:])

            if debug:
                nc.sync.dma_start(out=dbg_xa[:], in_=x_attn[:])
            # ================= SimAM =====================================
            nc.vector.reduce_sum(mub[:, 0:1], mu_parts[:, 0:NB2], axis=AX.X)
            nc.vector.reduce_sum(mub[:, 1:2], mu_parts[:, NB2:2 * NB2], axis=AX.X)
            nc.vector.tensor_scalar(out=mub[:], in0=mub[:],
                                    scalar1=-1.0 / N, scalar2=None, op0=OP.mult)
            nc.sync.dma_start(out=mu_neg[0:48, :], in_=mub[:, 0:1])
            nc.sync.dma_start(out=mu_neg[48:96, :], in_=mub[:, 1:2])
            for j in range(NB2):
                blk = slice(j * BLK2, (j + 1) * BLK2)
                nc.scalar.activation(d2[:, blk], x_attn[:, blk], AF.Square,
                                     bias=mu_neg[:], scale=1.0,
                                     accum_out=d2_parts[:, j:j + 1])
            nc.vector.reduce_sum(sden[:], d2_parts[:], axis=AX.X)
            nc.vector.tensor_scalar(out=sden[:], in0=sden[:],
                                    scalar1=4.0 / (N - 1), scalar2=4.0 * E_LAMBDA,
                                    op0=OP.mult, op1=OP.add)
            nc.vector.reciprocal_approx_fast(out=s_ch[:], in_=sden[:])
            for j in range(NB2):
                blk = slice(j * BLK2, (j + 1) * BLK2)
                nc.scalar.activation(sig[0:96, blk], d2[:, blk], AF.Sigmoid,
                                     bias=half_s[:], scale=s_ch[:])
                ob = pout.tile([96, BLK2], F32, tag="ob")
                nc.gpsimd.tensor_tensor(out=ob[:], in0=x_attn[:, blk],
                                        in1=sig[0:96, blk], op=OP.mult)
                nc.sync.dma_start(out=out_d[:, blk], in_=ob[:])

    nc.compile()
    return nc


_NC = None


def _get_nc():
    global _NC
    if _NC is None:
        _install_ntff_hook()
        _NC = build_nc()
    return _NC


def make_core_inputs(x, w_qkv, w_dw, temperature):
    """Host-side shard prep. Returns list of 8 input dicts."""
    x = np.asarray(x)
    w_qkv = np.asarray(w_qkv)
    w_dw = np.asarray(w_dw)
    temperature = np.asarray(temperature).reshape(8)
    in_maps = []
    for core in range(8):
        b, hg = core // 2, core % 2
        rows = np.concatenate([
            np.arange(hg * 96, hg * 96 + 96),           # q
            192 + np.arange(hg * 96, hg * 96 + 96),     # k
            384 + np.arange(hg * 96, hg * 96 + 96),     # v
            576 + np.arange(hg * 96, hg * 96 + 96),     # a
        ])
        W1 = w_qkv[rows, :, 0, 0]                        # [384, 192]
        W1T = np.ascontiguousarray(W1.T).astype(np.float16)
        wd9 = w_dw[rows, 0].reshape(384, 9).astype(np.float32)
        wdiag_h = np.zeros((128, NDIAG * 128), np.float16)
        wtap_h = np.zeros((128, 27), np.float32)
        for s in range(3):
            for t in range(9):
                wtap_h[:, s * 9 + t] = wd9[s * 128:(s + 1) * 128, t]
        for (s, dy, dx), idx in WDIAG_SLOT.items():
            t = (dy + 1) * 3 + (dx + 1)
            wdiag_h[np.arange(128), idx * 128 + np.arange(128)] = \
                wd9[s * 128:(s + 1) * 128, t].astype(np.float16)
        pat_h = np.zeros((128, 240), np.float16)
        for h in range(4):
            pat_h[h * 24:(h + 1) * 24, h * 24:(h + 1) * 24] = 1    # ones_q
        # ones_kA (rows 0:32 of cols 96:192): k-ch 0:32 -> rep cols
        pat_h[0:24, 96 + 0:96 + 24] = 1
        pat_h[24:32, 96 + 24:96 + 48] = 1
        # ones_kB (rows 32:96 of cols 96:192): k-ch 32:96
        pat_h[32 + 0:32 + 16, 96 + 24:96 + 48] = 1
        pat_h[32 + 16:32 + 40, 96 + 48:96 + 72] = 1
        pat_h[32 + 40:32 + 64, 96 + 72:96 + 96] = 1
        # D1-rep ones: cols 192:216 (rows 0:64), cols 216:240 (rows 64:128)
        pat_h[0:64, 192:216] = 1
        pat_h[64:128, 216:240] = 1
        heads = np.arange(hg * 4, hg * 4 + 4)
        t4 = temperature[heads].astype(np.float32)
        in_maps.append({
            "xin": x[b].reshape(192, N).astype(np.float16),
            "w1a": W1T[0:96].copy(),
            "w1b": W1T[96:192].copy(),
            "wdiag": wdiag_h,
            "wtap": wtap_h,
            "tmp0": np.repeat(t4[0:2], 24).reshape(48, 1).copy(),
            "tmp1": np.repeat(t4[2:4], 24).reshape(48, 1).copy(),
            "pat": pat_h,
        })
    return in_maps


def _assemble(results):
    full = np.empty((B, C, H, W), np.float32)
    for core in range(8):
        b, hg = core // 2, core % 2
        full[b, hg * 96:(hg + 1) * 96] = results[core]["out"].reshape(96, H, W)
    return full


def kernel(x, w_qkv, w_dw, temperature):
    nc = _get_nc()
    in_maps = make_core_inputs(x, w_qkv, w_dw, temperature)
    res = run_bass_kernel_spmd(nc, in_maps, list(range(8)))
    return _assemble(res.results)


def kernel_profiled(x, w_qkv, w_dw, temperature):
    nc = _get_nc()
    in_maps = make_core_inputs(x, w_qkv, w_dw, temperature)
    res = run_bass_kernel_spmd(nc, in_maps, list(range(8)), trace=True)
    return _assemble(res.results), res.exec_time_ns



# revision 20
# speedup vs baseline: 2.3044x; 2.3044x over previous
"""MASA agent-attention kernel for Trainium2, 8-core SPMD.

Sharding: core = (batch b in 0..3) x (head-group hg in 0..1).
Each core computes conv1x1 + depthwise3x3 for its 4 heads' q/k/v/a
channels (384 of 768), the agent attention for those heads, and SimAM
over its 96 output channels. No cross-core communication.

Per-core channel order: [q(96), k(96), a(0:64), v(96), a(64:96)].
SBUF slabs of 128: s0 = q[0:96]+k[0:32], s1 = k[32:96]+a[0:64],
s2 = v[0:96]+a[64:96].  v at slab base 0 so the v-transpose is one
[96,128] PE matmul (vs identity) per 128-pixel chunk.

Engine-op partition windows must be 32-aligned and (base==0 or count<=32).
"""

import sys
import types
import numpy as np

import concourse.bacc as bacc
import concourse.bass as bass
import concourse.mybir as mybir
from concourse.tile import TileContext
from concourse.bass_utils import run_bass_kernel_spmd

F16 = mybir.dt.float16
F32 = mybir.dt.float32
AX = mybir.AxisListType
OP = mybir.AluOpType
AF = mybir.ActivationFunctionType

B, C, H, W = 4, 192, 128, 128
N = H * W              # 16384
M_AG = 64              # agent tokens
E_LAMBDA = 1e-4
RS = 130               # padded row stride for pre
PREFREE = RS * RS      # 16900

TAPS = [(dy, dx) for dy in (-1, 0, 1) for dx in (-1, 0, 1)]
# tap offset in pre: (1+dy)*RS + (1+dx); odd offsets (dx==0) are
# 4B-misaligned for fp16 2x mode -> always on PE. Extra PE taps for balance.
PE_TAPS = {
    0: TAPS,                                  # slab0 fully on PE
    1: [t for t in TAPS if t[1] == 0],        # center column
    2: [t for t in TAPS if t[1] == 0],
}
DVE_TAPS = {s: [t for t in TAPS if t not in PE_TAPS[s]] for s in range(3)}
WDIAG_SLOT = {}
for _s in range(3):
    for _t in PE_TAPS[_s]:
        WDIAG_SLOT[(_s, _t[0], _t[1])] = len(WDIAG_SLOT)
NDIAG = len(WDIAG_SLOT)

NB2 = 16               # block count for norm / attention / simam phases
BLK2 = 1024
NCH = 128              # s-chunks of 128 for k-side


def _install_ntff_hook():
    try:
        import antenv.axon_hooks  # noqa: F401
        return
    except ImportError:
        pass
    try:
        from trn_agent_boot.trn_boot import _ntff_profile_via_ctypes
        hook = _ntff_profile_via_ctypes('/opt/axon/libaxon_pjrt.so')
        mod = types.ModuleType("antenv.axon_hooks")
        mod.get_axon_ntff_profile_hook = lambda: hook
        mod.set_axon_ntff_profile_hook = lambda h: None
        sys.modules["antenv.axon_hooks"] = mod
    except Exception:
        pass


def build_nc(debug=False):
    nc = bacc.Bacc("TRN2", target_bir_lowering=False, debug=False, num_devices=8)

    # ---- DRAM I/O ----
    xin = nc.dram_tensor("xin", [192, N], F16, kind="ExternalInput").ap()
    w1a = nc.dram_tensor("w1a", [96, 384], F16, kind="ExternalInput").ap()
    w1b = nc.dram_tensor("w1b", [96, 384], F16, kind="ExternalInput").ap()
    wdiag = nc.dram_tensor("wdiag", [128, NDIAG * 128], F16, kind="ExternalInput").ap()
    wtap = nc.dram_tensor("wtap", [128, 27], F32, kind="ExternalInput").ap()
    tmp0 = nc.dram_tensor("tmp0", [48, 1], F32, kind="ExternalInput").ap()
    tmp1 = nc.dram_tensor("tmp1", [48, 1], F32, kind="ExternalInput").ap()
    pat = nc.dram_tensor("pat", [128, 336], F16, kind="ExternalInput").ap()
    out_d = nc.dram_tensor("out", [96, N], F32, kind="ExternalOutput").ap()
    if debug:
        dbg_pre = nc.dram_tensor("dbg_pre", [128, PREFREE], F16, kind="ExternalOutput").ap()
        dbg_q = nc.dram_tensor("dbg_q", [128, N], F16, kind="ExternalOutput").ap()
        dbg_k = nc.dram_tensor("dbg_k", [128, N], F16, kind="ExternalOutput").ap()
        dbg_qn = nc.dram_tensor("dbg_qn", [128, N], F16, kind="ExternalOutput").ap()
        dbg_ag = nc.dram_tensor("dbg_ag", [96, 256], F16, kind="ExternalOutput").ap()
        dbg_av0 = nc.dram_tensor("dbg_av0", [128, 48], F16, kind="ExternalOutput").ap()
        dbg_av1 = nc.dram_tensor("dbg_av1", [128, 48], F16, kind="ExternalOutput").ap()
        dbg_xa = nc.dram_tensor("dbg_xa", [96, N], F16, kind="ExternalOutput").ap()
        dbg_vt = nc.dram_tensor("dbg_vt", [128, 98 * 4], F16, kind="ExternalOutput").ap()
        dbg_avi = nc.dram_tensor("dbg_avi", [128, 48], F16, kind="ExternalOutput").ap()
        dbg_e1 = nc.dram_tensor("dbg_e1", [128, BLK2], F16, kind="ExternalOutput").ap()
        dbg_op = nc.dram_tensor("dbg_op", [128, BLK2], F32, kind="ExternalOutput").ap()
        dbg_rqs = nc.dram_tensor("dbg_rqs", [48, BLK2], F32, kind="ExternalOutput").ap()

    # ---- persistent SBUF ----
    scratch = nc.alloc_sbuf_tensor("scratch", [128, PREFREE], F16).ap()
    dw0 = nc.alloc_sbuf_tensor("dw0", [128, N], F16).ap()
    dw1 = nc.alloc_sbuf_tensor("dw1", [128, N], F16).ap()
    dw2 = nc.alloc_sbuf_tensor("dw2", [128, N], F16).ap()
    dws = [dw0, dw1, dw2]
    w1a_s = nc.alloc_sbuf_tensor("w1a_s", [96, 384], F16).ap()
    w1b_s = nc.alloc_sbuf_tensor("w1b_s", [96, 384], F16).ap()
    wdiag_s = nc.alloc_sbuf_tensor("wdiag_s", [128, NDIAG * 128], F16).ap()
    wtap_s = nc.alloc_sbuf_tensor("wtap_s", [128, 27], F32).ap()
    ones_q = nc.alloc_sbuf_tensor("ones_q", [96, 96], F16).ap()
    ones_kA = nc.alloc_sbuf_tensor("ones_kA", [32, 96], F16).ap()
    ones_kB = nc.alloc_sbuf_tensor("ones_kB", [64, 96], F16).ap()
    ag_full = nc.alloc_sbuf_tensor("ag_full", [96, 256], F16).ap()
    agf = nc.alloc_sbuf_tensor("agf", [96, M_AG], F32).ap()
    agfs = nc.alloc_sbuf_tensor("agfs", [96, M_AG], F16).ap()
    temp_rep = nc.alloc_sbuf_tensor("temp_rep", [96, 1], F32).ap()
    av_l0 = nc.alloc_sbuf_tensor("av_l0", [128, 48], F16).ap()
    av_l1 = nc.alloc_sbuf_tensor("av_l1", [128, 48], F16).ap()
    dv_ones = nc.alloc_sbuf_tensor("dv_ones", [128, 48], F16).ap()
    idmat = nc.alloc_sbuf_tensor("idmat", [96, 96], F16).ap()
    asum = nc.alloc_sbuf_tensor("asum", [128, 2 * M_AG], F32).ap()  # rows 64:128
    rq2a = nc.alloc_sbuf_tensor("rq2a", [128, 1], F32).ap()
    rq2b = nc.alloc_sbuf_tensor("rq2b", [128, 1], F32).ap()
    mu_parts = nc.alloc_sbuf_tensor("mu_parts", [48, 2 * NB2], F32).ap()
    d2_parts = nc.alloc_sbuf_tensor("d2_parts", [96, NB2], F32).ap()
    mu_neg = nc.alloc_sbuf_tensor("mu_neg", [96, 1], F32).ap()
    mub = nc.alloc_sbuf_tensor("mub", [48, 2], F32).ap()
    sden = nc.alloc_sbuf_tensor("sden", [96, 1], F32).ap()
    s_ch = nc.alloc_sbuf_tensor("s_ch", [96, 1], F32).ap()
    half_s = nc.alloc_sbuf_tensor("half_s", [96, 1], F32).ap()

    # aliases (sequential reuse of big buffers)
    pre3 = scratch.rearrange("p (y x) -> p y x", x=RS)   # padded conv out
    vT = scratch[:, 0:NCH * 98]                          # after dwconv
    sig = scratch[:, 0:N]                                # final sigmoid (rows 0:96)
    as1 = scratch[:, 12544:12544 + 4096].bitcast(F32)    # [128,2048] pool stage1
    x_attn = dw1[0:96, :]                                # [96, N] f16 (phase D)
    kfull = dw2[0:96, :]                                 # k-hat packed (phase B)
    d2 = dw2[0:96, :]                                    # (phase E, after B)

    with TileContext(nc) as tc:
        with (
            tc.tile_pool(name="xio", bufs=4) as xio,
            tc.tile_pool(name="pout", bufs=2) as pout,
            tc.tile_pool(name="work", bufs=2) as work,
            tc.tile_pool(name="work1", bufs=1) as work1,
            tc.tile_pool(name="ppsum", bufs=2, space="PSUM") as ppsum,
        ):
            # ================= init =================
            nc.sync.dma_start(out=w1a_s[:], in_=w1a[:])
            nc.sync.dma_start(out=w1b_s[:], in_=w1b[:])
            nc.sync.dma_start(out=wdiag_s[:], in_=wdiag[:])
            nc.sync.dma_start(out=wtap_s[:], in_=wtap[:])
            # static patterns
            nc.sync.dma_start(out=ones_q[:], in_=pat[0:96, 0:96])
            nc.sync.dma_start(out=ones_kA[:], in_=pat[0:32, 96:192])
            nc.sync.dma_start(out=ones_kB[:], in_=pat[32:96, 96:192])
            nc.gpsimd.memset(av_l0[:], 0.0)
            nc.gpsimd.memset(av_l1[:], 0.0)
            # D1-rep ones lhsT: col j<24 -> even head (rows 0:64),
            # j>=24 -> odd head (rows 64:128)
            nc.sync.dma_start(out=dv_ones[:, 0:24], in_=pat[:, 192:216])
            nc.sync.dma_start(out=dv_ones[:, 24:48], in_=pat[:, 216:240])
            nc.sync.dma_start(out=idmat[:], in_=pat[0:96, 240:336])
            nc.gpsimd.memset(ag_full[:], 0.0)
            nc.sync.dma_start(out=temp_rep[0:48, :], in_=tmp0[:])
            nc.sync.dma_start(out=temp_rep[48:96, :], in_=tmp1[:])
            nc.gpsimd.memset(half_s[:], 0.5)
            # pre borders (rows 0 and 129, cols 0 and 129)
            nc.gpsimd.memset(pre3[:, 0, :], 0.0)
            nc.gpsimd.memset(pre3[:, 129, :], 0.0)
            nc.gpsimd.memset(pre3[:, :, 0], 0.0)
            nc.gpsimd.memset(pre3[:, :, 129], 0.0)

            if debug:
                nc.sync.dma_start(out=dbg_avi[:], in_=dv_ones[:])
            # ================= sweep1: conv1x1 + dwconv ====
            for s in range(3):
                wa = w1a_s[:, s * 128:(s + 1) * 128]
                wb = w1b_s[:, s * 128:(s + 1) * 128]
                nblk = N // 1024  # 16 blocks of 1024 (8 y-rows)

                def conv_blk(j, s=s, wa=wa, wb=wb):
                    x0 = xio.tile([96, 1024], F16, tag="x")
                    x1 = xio.tile([96, 1024], F16, tag="x")
                    nc.sync.dma_start(out=x0[:], in_=xin[0:96, j * 1024:(j + 1) * 1024])
                    nc.sync.dma_start(out=x1[:], in_=xin[96:192, j * 1024:(j + 1) * 1024])
                    ps = ppsum.tile([128, 1024], F32, tag="pA")
                    for q in range(2):
                        sl = slice(q * 512, (q + 1) * 512)
                        nc.tensor.matmul(ps[:, sl], wa, x0[:, sl], start=True, stop=False)
                        nc.tensor.matmul(ps[:, sl], wb, x1[:, sl], start=False, stop=True)
                    nc.scalar.copy(pre3[:, 1 + 8 * j: 9 + 8 * j, 1:129], ps[:])

                def dw_blk(j, s=s):
                    dst = dws[s][:, j * 1024:(j + 1) * 1024]
                    pe_t = PE_TAPS[s]
                    dv_t = DVE_TAPS[s]
                    pd = None
                    if pe_t:
                        pd = ppsum.tile([128, 1024], F32, tag="pB")
                        for q in range(2):
                            for ti, (dy, dx) in enumerate(pe_t):
                                dg = wdiag_s[:, WDIAG_SLOT[(s, dy, dx)] * 128:
                                             (WDIAG_SLOT[(s, dy, dx)] + 1) * 128]
                                rv = pre3[:, 1 + dy + 8 * j + 4 * q: 5 + dy + 8 * j + 4 * q,
                                          1 + dx: 129 + dx]
                                nc.tensor.matmul(pd[:, q * 512:(q + 1) * 512], dg, rv,
                                                 start=(ti == 0), stop=(ti == len(pe_t) - 1))
                    first = True
                    for (dy, dx) in dv_t:
                        ti = s * 9 + TAPS.index((dy, dx))
                        w_sc = wtap_s[:, ti:ti + 1]
                        rv = pre3[:, 1 + dy + 8 * j: 9 + dy + 8 * j, 1 + dx: 129 + dx]
                        if first and pd is not None:
                            nc.vector.scalar_tensor_tensor(
                                out=dst, in0=rv, scalar=w_sc, in1=pd[:],
                                op0=OP.mult, op1=OP.add)
                        elif first:
                            nc.vector.tensor_scalar(
                                out=dst, in0=rv, scalar1=w_sc, scalar2=None, op0=OP.mult)
                        else:
                            nc.vector.scalar_tensor_tensor(
                                out=dst, in0=rv, scalar=w_sc, in1=dst,
                                op0=OP.mult, op1=OP.add)
                        first = False
                    if not dv_t:
                        nc.scalar.copy(dst, pd[:])

                conv_blk(0)
                for j in range(1, nblk):
                    conv_blk(j)
                    dw_blk(j - 1)
                dw_blk(nblk - 1)

            if debug:
                nc.sync.dma_start(out=dbg_pre[:], in_=scratch[:])
                nc.sync.dma_start(out=dbg_q[:], in_=dw0[:])
                nc.sync.dma_start(out=dbg_k[:], in_=dw1[:])
            # ===== agent pooling (a = dw1[64:96], dw1[96:128], dw2[96:128])
            AGRP = ((dw1, 64, 0), (dw1, 96, 0), (dw2, 96, 1))
            for (abuf, w0, half) in AGRP:
                a3 = abuf[w0:w0 + 32, :].rearrange("p (a xi) -> p a xi", xi=16)
                s1 = as1[w0:w0 + 32, half * 1024:(half + 1) * 1024]
                nc.vector.reduce_sum(s1, a3, axis=AX.X)
                as3 = s1.rearrange("p (yb yi xb) -> p yb xb yi",
                                   yb=8, yi=16, xb=8)
                asum3 = asum[w0:w0 + 32, half * 64:(half + 1) * 64].rearrange(
                    "p (yb xb) -> p yb xb", yb=8)
                nc.vector.reduce_sum(asum3, as3, axis=AX.X)
            nc.sync.dma_start(out=agf[0:32, :], in_=asum[64:96, 0:64])
            nc.sync.dma_start(out=agf[32:64, :], in_=asum[96:128, 0:64])
            nc.sync.dma_start(out=agf[64:96, :], in_=asum[96:128, 64:128])

            # ============ vT build via PE transpose (into scratch) =======
            # per-chunk layout [one | v-ch 0:96 | one]: cols 0, 1:97, 97
            # 8 chunks (24 transposes of [32,128] -> [128,32] f16 in PSUM)
            # per ACT copy into the strided vT3 slots.
            vT3 = vT.rearrange("p (c w) -> p c w", w=98)
            nc.gpsimd.memset(vT3[:, :, 0], 1.0)
            nc.gpsimd.memset(vT3[:, :, 97], 1.0)
            for c0 in range(0, NCH, 5):
                cs = min(5, NCH - c0)
                pt = ppsum.tile([128, 512], F32, tag="pA", name="pt")
                for ci in range(cs):
                    ssl = slice((c0 + ci) * 128, (c0 + ci + 1) * 128)
                    nc.tensor.matmul(
                        pt[:, ci * 96:ci * 96 + 96],
                        dw2[0:96, ssl], idmat[:],
                        start=True, stop=True)
                nc.scalar.copy(vT3[:, c0:c0 + cs, 1:97], pt[:, 0:cs * 96])
            # scale by temp/256 (per-partition scalar), then place blocks by DMA
            nc.vector.tensor_scalar(out=agfs[:], in0=agf[:],
                                    scalar1=temp_rep[:], scalar2=1.0 / 256.0,
                                    op0=OP.mult, op1=OP.mult)
            for h in range(4):
                nc.sync.dma_start(
                    out=ag_full[h * 24:(h + 1) * 24, h * 64:(h + 1) * 64],
                    in_=agfs[h * 24:(h + 1) * 24, :])

            # ================= l2norm of q, k =========================
            for j in range(NB2):
                blk = slice(j * BLK2, (j + 1) * BLK2)
                sq0 = work1.tile([128, BLK2], F16, tag="sq0")
                sq1 = work1.tile([64, BLK2], F16, tag="sq1")
                sqk = work1.tile([32, BLK2], F16, tag="sqk")
                nc.gpsimd.tensor_tensor(out=sq0[:], in0=dw0[:, blk], in1=dw0[:, blk],
                                        op=OP.mult)
                nc.gpsimd.tensor_tensor(out=sq1[:], in0=dw1[0:64, blk],
                                        in1=dw1[0:64, blk], op=OP.mult)
                nc.sync.dma_start(out=sqk[:], in_=sq0[96:128, :])
                pq = ppsum.tile([96, BLK2], F32, tag="pA")
                pk = ppsum.tile([96, BLK2], F32, tag="pB")
                for q in range(2):
                    sl = slice(q * 512, (q + 1) * 512)
                    nc.tensor.matmul(pq[:, sl], ones_q[:], sq0[0:96, sl],
                                     start=True, stop=True)
                    nc.tensor.matmul(pk[:, sl], ones_kA[:], sqk[:, sl],
                                     start=True, stop=False)
                    nc.tensor.matmul(pk[:, sl], ones_kB[:], sq1[:, sl],
                                     start=False, stop=True)
                rns_q = work1.tile([96, BLK2], F32, tag="rns_q")
                rns_k = work1.tile([96, BLK2], F32, tag="rns_k")
                nc.vector.reciprocal_approx_fast(out=rns_q[:], in_=pq[:])
                nc.vector.reciprocal_approx_fast(out=rns_k[:], in_=pk[:])
                rinv_q = work1.tile([96, BLK2], F16, tag="rinv_q")
                rinv_k = work1.tile([96, BLK2], F16, tag="rinv_k")
                nc.scalar.activation(rinv_q[:], rns_q[:], AF.Sqrt)
                nc.scalar.activation(rinv_k[:], rns_k[:], AF.Sqrt)
                rrk = work1.tile([128, BLK2], F16, tag="rrk")
                nc.sync.dma_start(out=rrk[96:128, :], in_=rinv_k[0:32, :])
                nc.sync.dma_start(out=rrk[0:64, :], in_=rinv_k[32:96, :])
                nc.gpsimd.tensor_tensor(out=dw0[0:96, blk], in0=dw0[0:96, blk],
                                        in1=rinv_q[:], op=OP.mult)
                nc.gpsimd.tensor_tensor(out=dw0[96:128, blk], in0=dw0[96:128, blk],
                                        in1=rrk[96:128, :], op=OP.mult)
                nc.gpsimd.tensor_tensor(out=dw1[0:64, blk], in0=dw1[0:64, blk],
                                        in1=rrk[0:64, :], op=OP.mult)

            if debug:
                nc.sync.dma_start(out=dbg_qn[:], in_=dw0[:])
            # ====== pack k-hat contiguous at base 0 (into dw2 rows 0:96) ==
            # (v rows of dw2 already consumed by vT; a rows by pooling)
            nc.sync.dma_start(out=kfull[0:32, :], in_=dw0[96:128, :])
            nc.sync.dma_start(out=kfull[32:96, :], in_=dw1[0:64, :])

            # ================= k-side: L2T -> exp -> agent_v =============
            agv0 = ppsum.tile([128, 49], F32, tag="pB")
            agv1 = ppsum.tile([128, 49], F32, tag="pB")
            for c in range(NCH):
                ssl = slice(c * 128, (c + 1) * 128)
                l2 = ppsum.tile([128, 256], F32, tag="pA")
                nc.tensor.matmul(l2[:], kfull[:, ssl], ag_full[:],
                                 start=True, stop=True)
                e2t = work.tile([128, 256], F16, tag="e2t")
                nc.scalar.activation(e2t[:], l2[:], AF.Exp)
                nc.tensor.matmul(agv0[:], e2t[:, 0:128], vT3[:, c, 0:49],
                                 start=(c == 0), stop=(c == NCH - 1))
                nc.tensor.matmul(agv1[:], e2t[:, 128:256], vT3[:, c, 49:98],
                                 start=(c == 0), stop=(c == NCH - 1))
            # agv0: D2 col 0, channels cols 1:49. agv1: channels 0:48, D2 col 48.
            nc.vector.reciprocal_approx_fast(out=rq2a[:], in_=agv0[:, 0:1])
            nc.vector.reciprocal_approx_fast(out=rq2b[:], in_=agv1[:, 48:49])
            # block-diagonal: even head of pair -> rows 0:64 x cols 0:24,
            # odd head -> rows 64:128 x cols 24:48 (other entries stay zero)
            nc.vector.tensor_scalar(out=av_l0[0:64, 0:24], in0=agv0[0:64, 1:25],
                                    scalar1=rq2a[0:64, :], scalar2=None, op0=OP.mult)
            for w0 in (64, 96):
                nc.vector.tensor_scalar(out=av_l0[w0:w0 + 32, 24:48],
                                        in0=agv0[w0:w0 + 32, 25:49],
                                        scalar1=rq2a[w0:w0 + 32, :], scalar2=None,
                                        op0=OP.mult)
            nc.vector.tensor_scalar(out=av_l1[0:64, 0:24], in0=agv1[0:64, 0:24],
                                    scalar1=rq2b[0:64, :], scalar2=None, op0=OP.mult)
            for w0 in (64, 96):
                nc.vector.tensor_scalar(out=av_l1[w0:w0 + 32, 24:48],
                                        in0=agv1[w0:w0 + 32, 24:48],
                                        scalar1=rq2b[w0:w0 + 32, :], scalar2=None,
                                        op0=OP.mult)

            if debug:
                nc.sync.dma_start(out=dbg_ag[:], in_=ag_full[:])
                nc.sync.dma_start(out=dbg_av0[:], in_=av_l0[:])
                nc.sync.dma_start(out=dbg_av1[:], in_=av_l1[:])
                nc.sync.dma_start(out=dbg_vt[:], in_=vT[:, 0:98 * 4])
            # ================= q-side + division =========================
            # out' psum rows: 0:48 channels, 64:88 D1-rep (even head of pair),
            # 96:120 D1-rep (odd head of pair)
            for hp in range(2):
                av_l = av_l0 if hp == 0 else av_l1
                ag_cols = ag_full[:, hp * 128:(hp + 1) * 128]
                for j in range(NB2):
                    blk = slice(j * BLK2, (j + 1) * BLK2)
                    l1 = ppsum.tile([128, BLK2], F32, tag="pA")
                    for q in range(2):
                        sl = slice(j * BLK2 + q * 512, j * BLK2 + (q + 1) * 512)
                        psl = slice(q * 512, (q + 1) * 512)
                        nc.tensor.matmul(l1[:, psl], ag_cols, dw0[0:96, sl],
                                         start=True, stop=True)
                    e1 = work.tile([128, BLK2], F16, tag="e1")
                    nc.scalar.activation(e1[:], l1[:], AF.Exp)
                    op_ = ppsum.tile([48, BLK2], F32, tag="pB")
                    od_ = ppsum.tile([48, BLK2], F32, tag="pB")
                    for q in range(2):
                        psl = slice(q * 512, (q + 1) * 512)
                        nc.tensor.matmul(op_[:, psl], av_l[:], e1[:, psl],
                                         start=True, stop=True)
                        nc.tensor.matmul(od_[:, psl], dv_ones[:], e1[:, psl],
                                         start=True, stop=True)
                    rqs = work1.tile([48, BLK2], F32, tag="rqs")
                    nc.vector.reciprocal_approx_fast(out=rqs[:], in_=od_[:])
                    if debug and hp == 0 and j == 0:
                        nc.sync.dma_start(out=dbg_e1[:], in_=e1[:])
                        opc = work1.tile([48, BLK2], F32, tag="opc")
                        nc.scalar.copy(opc[:], op_[:])
                        nc.sync.dma_start(out=dbg_op[0:48, :], in_=opc[:])
                        nc.sync.dma_start(out=dbg_rqs[:], in_=rqs[:])
                    if hp == 0:
                        nc.vector.scalar_tensor_tensor(
                            out=x_attn[0:48, blk], in0=op_[:], scalar=0.0,
                            in1=rqs[:], op0=OP.bypass, op1=OP.mult,
                            accum_out=mu_parts[:, j:j + 1])
                    else:
                        xt = work1.tile([48, BLK2], F16, tag="xt")
                        nc.vector.scalar_tensor_tensor(
                            out=xt[:], in0=op_[:], scalar=0.0,
                            in1=rqs[:], op0=OP.bypass, op1=OP.mult,
                            accum_out=mu_parts[:, NB2 + j:NB2 + j + 1])
                        nc.sync.dma_start(out=dw1[48:96, blk], in_=xt[:])

            if debug:
                nc.sync.dma_start(out=dbg_xa[:], in_=x_attn[:])
            # ================= SimAM =====================================
            nc.vector.reduce_sum(mub[:, 0:1], mu_parts[:, 0:NB2], axis=AX.X)
            nc.vector.reduce_sum(mub[:, 1:2], mu_parts[:, NB2:2 * NB2], axis=AX.X)
            nc.vector.tensor_scalar(out=mub[:], in0=mub[:],
                                    scalar1=-1.0 / N, scalar2=None, op0=OP.mult)
            nc.sync.dma_start(out=mu_neg[0:48, :], in_=mub[:, 0:1])
            nc.sync.dma_start(out=mu_neg[48:96, :], in_=mub[:, 1:2])
            for j in range(NB2):
                blk = slice(j * BLK2, (j + 1) * BLK2)
                nc.scalar.activation(d2[:, blk], x_attn[:, blk], AF.Square,
                                     bias=mu_neg[:], scale=1.0,
                                     accum_out=d2_parts[:, j:j + 1])
            nc.vector.reduce_sum(sden[:], d2_parts[:], axis=AX.X)
            nc.vector.tensor_scalar(out=sden[:], in0=sden[:],
                                    scalar1=4.0 / (N - 1), scalar2=4.0 * E_LAMBDA,
                                    op0=OP.mult, op1=OP.add)
            nc.vector.reciprocal_approx_fast(out=s_ch[:], in_=sden[:])
            for j in range(NB2):
                blk = slice(j * BLK2, (j + 1) * BLK2)
                nc.scalar.activation(sig[0:96, blk], d2[:, blk], AF.Sigmoid,
                                     bias=half_s[:], scale=s_ch[:])
                ob = pout.tile([96, BLK2], F32, tag="ob")
                nc.gpsimd.tensor_tensor(out=ob[:], in0=x_attn[:, blk],
                                        in1=sig[0:96, blk], op=OP.mult)
                nc.sync.dma_start(out=out_d[:, blk], in_=ob[:])

    nc.compile()
    return nc


_NC = None


def _get_nc():
    global _NC
    if _NC is None:
        _install_ntff_hook()
        _NC = build_nc()
    return _NC


def make_core_inputs(x, w_qkv, w_dw, temperature):
    """Host-side shard prep. Returns list of 8 input dicts."""
    x = np.asarray(x)
    w_qkv = np.asarray(w_qkv)
    w_dw = np.asarray(w_dw)
    temperature = np.asarray(temperature).reshape(8)
    in_maps = []
    for core in range(8):
        b, hg = core // 2, core % 2
        # slab0 = q + k[0:32]; slab1 = k[32:96] + a[0:64];
        # slab2 = v[0:96] + a[64:96]  (v at base 0 for PE transpose)
        rows = np.concatenate([
            np.arange(hg * 96, hg * 96 + 96),           # q
            192 + np.arange(hg * 96, hg * 96 + 96),     # k
            576 + np.arange(hg * 96, hg * 96 + 64),     # a[0:64]
            384 + np.arange(hg * 96, hg * 96 + 96),     # v
            576 + np.arange(hg * 96 + 64, hg * 96 + 96),  # a[64:96]
        ])
        W1 = w_qkv[rows, :, 0, 0]                        # [384, 192]
        W1T = np.ascontiguousarray(W1.T).astype(np.float16)
        wd9 = w_dw[rows, 0].reshape(384, 9).astype(np.float32)
        wdiag_h = np.zeros((128, NDIAG * 128), np.float16)
        wtap_h = np.zeros((128, 27), np.float32)
        for s in range(3):
            for t in range(9):
                wtap_h[:, s * 9 + t] = wd9[s * 128:(s + 1) * 128, t]
        for (s, dy, dx), idx in WDIAG_SLOT.items():
            t = (dy + 1) * 3 + (dx + 1)
            wdiag_h[np.arange(128), idx * 128 + np.arange(128)] = \
                wd9[s * 128:(s + 1) * 128, t].astype(np.float16)
        pat_h = np.zeros((128, 336), np.float16)
        pat_h[np.arange(96), 240 + np.arange(96)] = 1   # I96 for v transpose
        for h in range(4):
            pat_h[h * 24:(h + 1) * 24, h * 24:(h + 1) * 24] = 1    # ones_q
        # ones_kA (rows 0:32 of cols 96:192): k-ch 0:32 -> rep cols
        pat_h[0:24, 96 + 0:96 + 24] = 1
        pat_h[24:32, 96 + 24:96 + 48] = 1
        # ones_kB (rows 32:96 of cols 96:192): k-ch 32:96
        pat_h[32 + 0:32 + 16, 96 + 24:96 + 48] = 1
        pat_h[32 + 16:32 + 40, 96 + 48:96 + 72] = 1
        pat_h[32 + 40:32 + 64, 96 + 72:96 + 96] = 1
        # D1-rep ones: cols 192:216 (rows 0:64), cols 216:240 (rows 64:128)
        pat_h[0:64, 192:216] = 1
        pat_h[64:128, 216:240] = 1
        heads = np.arange(hg * 4, hg * 4 + 4)
        t4 = temperature[heads].astype(np.float32)
        in_maps.append({
            "xin": x[b].reshape(192, N).astype(np.float16),
            "w1a": W1T[0:96].copy(),
            "w1b": W1T[96:192].copy(),
            "wdiag": wdiag_h,
            "wtap": wtap_h,
            "tmp0": np.repeat(t4[0:2], 24).reshape(48, 1).copy(),
            "tmp1": np.repeat(t4[2:4], 24).reshape(48, 1).copy(),
            "pat": pat_h,
        })
    return in_maps


def _assemble(results):
    full = np.empty((B, C, H, W), np.float32)
    for core in range(8):
        b, hg = core // 2, core % 2
        full[b, hg * 96:(hg + 1) * 96] = results[core]["out"].reshape(96, H, W)
    return full


def kernel(x, w_qkv, w_dw, temperature):
    nc = _get_nc()
    in_maps = make_core_inputs(x, w_qkv, w_dw, temperature)
    res = run_bass_kernel_spmd(nc, in_maps, list(range(8)))
    return _assemble(res.results)


def kernel_profiled(x, w_qkv, w_dw, temperature):
    nc = _get_nc()
    in_maps = make_core_inputs(x, w_qkv, w_dw, temperature)
    res = run_bass_kernel_spmd(nc, in_maps, list(range(8)), trace=True)
    return _assemble(res.results), res.exec_time_ns



# revision 27
# speedup vs baseline: 3.2731x; 1.4204x over previous
"""MASA agent-attention kernel for Trainium2, 8-core SPMD.

Sharding: core = (batch b in 0..3) x (head-group hg in 0..1).
Each core computes conv1x1 + depthwise3x3 for its 4 heads' q/k/v/a
channels (384 of 768), the agent attention for those heads, and SimAM
over its 96 output channels. No cross-core communication.

Per-core channel order: [q(96), k(96), a(0:64), v(96), a(64:96)].
SBUF slabs of 128: s0 = q[0:96]+k[0:32], s1 = k[32:96]+a[0:64],
s2 = v[0:96]+a[64:96].  v at slab base 0 so the v-transpose is one
[96,128] PE matmul (vs identity) per 128-pixel chunk.

Engine-op partition windows must be 32-aligned and (base==0 or count<=32).
"""

import sys
import types
import numpy as np

import concourse.bacc as bacc
import concourse.bass as bass
import concourse.mybir as mybir
from concourse.tile import TileContext
from concourse.bass_utils import run_bass_kernel_spmd

F16 = mybir.dt.float16
F32 = mybir.dt.float32
AX = mybir.AxisListType
OP = mybir.AluOpType
AF = mybir.ActivationFunctionType

B, C, H, W = 4, 192, 128, 128
N = H * W              # 16384
M_AG = 64              # agent tokens
E_LAMBDA = 1e-4
RS = 130               # padded row stride for pre
PREFREE = RS * RS      # 16900

TAPS = [(dy, dx) for dy in (-1, 0, 1) for dx in (-1, 0, 1)]
# tap offset in pre: (1+dy)*RS + (1+dx); odd offsets (dx==0) are
# 4B-misaligned for fp16 2x mode -> always on PE. DVE gets only the
# dx=+1 column (aligned), as tensor_scalar products + tensor_tensor
# adds (packed modes); scalar_tensor_tensor is always 1x on DVE.
PE_TAPS = {
    0: TAPS,                                  # slab0 fully on PE
    1: [t for t in TAPS if t[1] <= 0],        # dx in {-1, 0}
    2: [t for t in TAPS if t[1] <= 0],
}
DVE_TAPS = {s: [t for t in TAPS if t not in PE_TAPS[s]] for s in range(3)}
WDIAG_SLOT = {}
for _s in range(3):
    for _t in PE_TAPS[_s]:
        WDIAG_SLOT[(_s, _t[0], _t[1])] = len(WDIAG_SLOT)
NDIAG = len(WDIAG_SLOT)

NB2 = 16               # block count for norm / attention / simam phases
BLK2 = 1024
NCH = 128              # s-chunks of 128 for k-side


def _install_ntff_hook():
    try:
        import antenv.axon_hooks  # noqa: F401
        return
    except ImportError:
        pass
    try:
        from trn_agent_boot.trn_boot import _ntff_profile_via_ctypes
        hook = _ntff_profile_via_ctypes('/opt/axon/libaxon_pjrt.so')
        mod = types.ModuleType("antenv.axon_hooks")
        mod.get_axon_ntff_profile_hook = lambda: hook
        mod.set_axon_ntff_profile_hook = lambda h: None
        sys.modules["antenv.axon_hooks"] = mod
    except Exception:
        pass


def build_nc(debug=False):
    nc = bacc.Bacc("TRN2", target_bir_lowering=False, debug=False, num_devices=8)

    # ---- DRAM I/O ----
    xin = nc.dram_tensor("xin", [192, N], F16, kind="ExternalInput").ap()
    w1a = nc.dram_tensor("w1a", [96, 384], F16, kind="ExternalInput").ap()
    w1b = nc.dram_tensor("w1b", [96, 384], F16, kind="ExternalInput").ap()
    wdiag = nc.dram_tensor("wdiag", [128, NDIAG * 128], F16, kind="ExternalInput").ap()
    wtap = nc.dram_tensor("wtap", [128, 27], F32, kind="ExternalInput").ap()
    tmp0 = nc.dram_tensor("tmp0", [48, 1], F32, kind="ExternalInput").ap()
    tmp1 = nc.dram_tensor("tmp1", [48, 1], F32, kind="ExternalInput").ap()
    pat = nc.dram_tensor("pat", [128, 336], F16, kind="ExternalInput").ap()
    out_d = nc.dram_tensor("out", [96, N], F32, kind="ExternalOutput").ap()
    if debug:
        dbg_pre = nc.dram_tensor("dbg_pre", [128, PREFREE], F16, kind="ExternalOutput").ap()
        dbg_q = nc.dram_tensor("dbg_q", [128, N], F16, kind="ExternalOutput").ap()
        dbg_k = nc.dram_tensor("dbg_k", [128, N], F16, kind="ExternalOutput").ap()
        dbg_qn = nc.dram_tensor("dbg_qn", [128, N], F16, kind="ExternalOutput").ap()
        dbg_ag = nc.dram_tensor("dbg_ag", [96, 256], F16, kind="ExternalOutput").ap()
        dbg_av0 = nc.dram_tensor("dbg_av0", [128, 48], F16, kind="ExternalOutput").ap()
        dbg_av1 = nc.dram_tensor("dbg_av1", [128, 48], F16, kind="ExternalOutput").ap()
        dbg_xa = nc.dram_tensor("dbg_xa", [96, N], F16, kind="ExternalOutput").ap()
        dbg_vt = nc.dram_tensor("dbg_vt", [128, 98 * 4], F16, kind="ExternalOutput").ap()
        dbg_avi = nc.dram_tensor("dbg_avi", [128, 48], F16, kind="ExternalOutput").ap()
        dbg_e1 = nc.dram_tensor("dbg_e1", [128, BLK2], F16, kind="ExternalOutput").ap()
        dbg_op = nc.dram_tensor("dbg_op", [128, BLK2], F32, kind="ExternalOutput").ap()
        dbg_rqs = nc.dram_tensor("dbg_rqs", [48, BLK2], F32, kind="ExternalOutput").ap()

    # ---- persistent SBUF ----
    scratch = nc.alloc_sbuf_tensor("scratch", [128, PREFREE], F16).ap()
    dw0 = nc.alloc_sbuf_tensor("dw0", [128, N], F16).ap()
    dw1 = nc.alloc_sbuf_tensor("dw1", [128, N], F16).ap()
    dw2 = nc.alloc_sbuf_tensor("dw2", [128, N], F16).ap()
    dws = [dw0, dw1, dw2]
    w1a_s = nc.alloc_sbuf_tensor("w1a_s", [96, 384], F16).ap()
    w1b_s = nc.alloc_sbuf_tensor("w1b_s", [96, 384], F16).ap()
    wdiag_s = nc.alloc_sbuf_tensor("wdiag_s", [128, NDIAG * 128], F16).ap()
    wtap_s = nc.alloc_sbuf_tensor("wtap_s", [128, 27], F32).ap()
    ones_q = nc.alloc_sbuf_tensor("ones_q", [96, 96], F16).ap()
    ones_kA = nc.alloc_sbuf_tensor("ones_kA", [32, 96], F16).ap()
    ones_kB = nc.alloc_sbuf_tensor("ones_kB", [64, 96], F16).ap()
    ag_full = nc.alloc_sbuf_tensor("ag_full", [96, 256], F16).ap()
    agf = nc.alloc_sbuf_tensor("agf", [96, M_AG], F32).ap()
    agfs = nc.alloc_sbuf_tensor("agfs", [96, M_AG], F16).ap()
    temp_rep = nc.alloc_sbuf_tensor("temp_rep", [96, 1], F32).ap()
    av_l0 = nc.alloc_sbuf_tensor("av_l0", [128, 48], F16).ap()
    av_l1 = nc.alloc_sbuf_tensor("av_l1", [128, 48], F16).ap()
    dv_ones = nc.alloc_sbuf_tensor("dv_ones", [128, 48], F16).ap()
    idmat = nc.alloc_sbuf_tensor("idmat", [96, 96], F16).ap()
    asum = nc.alloc_sbuf_tensor("asum", [128, 2 * M_AG], F32).ap()  # rows 64:128
    rq2a = nc.alloc_sbuf_tensor("rq2a", [128, 1], F32).ap()
    rq2b = nc.alloc_sbuf_tensor("rq2b", [128, 1], F32).ap()
    mu_parts = nc.alloc_sbuf_tensor("mu_parts", [128, NB2], F32).ap()
    d2_parts = nc.alloc_sbuf_tensor("d2_parts", [128, NB2], F32).ap()
    mub = nc.alloc_sbuf_tensor("mub", [128, 1], F32).ap()
    sden = nc.alloc_sbuf_tensor("sden", [128, 1], F32).ap()
    s_ch = nc.alloc_sbuf_tensor("s_ch", [128, 1], F32).ap()
    half_s = nc.alloc_sbuf_tensor("half_s", [128, 1], F32).ap()

    # aliases (sequential reuse of big buffers)
    pre3 = scratch.rearrange("p (y x) -> p y x", x=RS)   # padded conv out
    vT = scratch[:, 0:NCH * 98]                          # after dwconv
    sig = scratch[:, 0:N]                                # final sigmoid (rows 0:96)
    as1 = scratch[:, 12544:12544 + 4096].bitcast(F32)    # [128,2048] pool stage1
    x_attn = dw1[:, :]                                   # [128, N] f16 (phase D)
    kfull = dw2[0:96, :]                                 # k-hat packed (phase B)
    d2 = dw2[:, :]                                       # (phase E, after B)

    with TileContext(nc) as tc:
        with (
            tc.tile_pool(name="xio", bufs=4) as xio,
            tc.tile_pool(name="pout", bufs=2) as pout,
            tc.tile_pool(name="work", bufs=2) as work,
            tc.tile_pool(name="work1", bufs=1) as work1,
            tc.tile_pool(name="ppsum", bufs=2, space="PSUM") as ppsum,
        ):
            # ================= init =================
            nc.sync.dma_start(out=w1a_s[:], in_=w1a[:])
            nc.sync.dma_start(out=w1b_s[:], in_=w1b[:])
            nc.sync.dma_start(out=wdiag_s[:], in_=wdiag[:])
            nc.sync.dma_start(out=wtap_s[:], in_=wtap[:])
            # static patterns
            nc.sync.dma_start(out=ones_q[:], in_=pat[0:96, 0:96])
            nc.sync.dma_start(out=ones_kA[:], in_=pat[0:32, 96:192])
            nc.sync.dma_start(out=ones_kB[:], in_=pat[32:96, 96:192])
            nc.gpsimd.memset(av_l0[:], 0.0)
            nc.gpsimd.memset(av_l1[:], 0.0)
            # D1-rep ones lhsT: col j<24 -> even head (rows 0:64),
            # j>=24 -> odd head (rows 64:128)
            nc.sync.dma_start(out=dv_ones[:, 0:24], in_=pat[:, 192:216])
            nc.sync.dma_start(out=dv_ones[:, 24:48], in_=pat[:, 216:240])
            nc.sync.dma_start(out=idmat[:], in_=pat[0:96, 240:336])
            nc.gpsimd.memset(ag_full[:], 0.0)
            nc.sync.dma_start(out=temp_rep[0:48, :], in_=tmp0[:])
            nc.sync.dma_start(out=temp_rep[48:96, :], in_=tmp1[:])
            nc.gpsimd.memset(half_s[:], 0.5)
            # pre borders (rows 0 and 129, cols 0 and 129)
            nc.gpsimd.memset(pre3[:, 0, :], 0.0)
            nc.gpsimd.memset(pre3[:, 129, :], 0.0)
            nc.gpsimd.memset(pre3[:, :, 0], 0.0)
            nc.gpsimd.memset(pre3[:, :, 129], 0.0)

            if debug:
                nc.sync.dma_start(out=dbg_avi[:], in_=dv_ones[:])
            # ================= sweep1: conv1x1 + dwconv ====
            for s in range(3):
                wa = w1a_s[:, s * 128:(s + 1) * 128]
                wb = w1b_s[:, s * 128:(s + 1) * 128]
                nblk = N // 1024  # 16 blocks of 1024 (8 y-rows)

                def conv_blk(j, s=s, wa=wa, wb=wb):
                    x0 = xio.tile([96, 1024], F16, tag="x")
                    x1 = xio.tile([96, 1024], F16, tag="x")
                    nc.sync.dma_start(out=x0[:], in_=xin[0:96, j * 1024:(j + 1) * 1024])
                    nc.sync.dma_start(out=x1[:], in_=xin[96:192, j * 1024:(j + 1) * 1024])
                    ps = ppsum.tile([128, 1024], F32, tag="pA")
                    for q in range(2):
                        sl = slice(q * 512, (q + 1) * 512)
                        nc.tensor.matmul(ps[:, sl], wa, x0[:, sl], start=True, stop=False)
                        nc.tensor.matmul(ps[:, sl], wb, x1[:, sl], start=False, stop=True)
                    nc.scalar.copy(pre3[:, 1 + 8 * j: 9 + 8 * j, 1:129], ps[:])

                def dw_blk(j, s=s):
                    dst = dws[s][:, j * 1024:(j + 1) * 1024]
                    pe_t = PE_TAPS[s]
                    dv_t = DVE_TAPS[s]
                    pd = None
                    if pe_t:
                        pd = ppsum.tile([128, 1024], F32, tag="pB")
                        for q in range(2):
                            for ti, (dy, dx) in enumerate(pe_t):
                                dg = wdiag_s[:, WDIAG_SLOT[(s, dy, dx)] * 128:
                                             (WDIAG_SLOT[(s, dy, dx)] + 1) * 128]
                                rv = pre3[:, 1 + dy + 8 * j + 4 * q: 5 + dy + 8 * j + 4 * q,
                                          1 + dx: 129 + dx]
                                nc.tensor.matmul(pd[:, q * 512:(q + 1) * 512], dg, rv,
                                                 start=(ti == 0), stop=(ti == len(pe_t) - 1))
                    if dv_t:
                        # 3 aligned taps: 1 STT (merges PE psum, 1x) +
                        # 2 TS products (4x) + 2 TT adds (2x)
                        def win(dy, dx):
                            return pre3[:, 1 + dy + 8 * j: 9 + dy + 8 * j,
                                        1 + dx: 129 + dx]

                        def wsc(dy, dx):
                            ti = s * 9 + TAPS.index((dy, dx))
                            return wtap_s[:, ti:ti + 1]

                        ta = work.tile([128, 1024], F16, tag="dta")
                        nc.vector.scalar_tensor_tensor(
                            out=ta[:], in0=win(*dv_t[0]), scalar=wsc(*dv_t[0]),
                            in1=pd[:], op0=OP.mult, op1=OP.add)
                        tb = work.tile([128, 1024], F16, tag="dtb")
                        nc.vector.tensor_scalar(
                            out=tb[:], in0=win(*dv_t[1]), scalar1=wsc(*dv_t[1]),
                            scalar2=None, op0=OP.mult)
                        nc.vector.tensor_scalar(
                            out=dst, in0=win(*dv_t[2]), scalar1=wsc(*dv_t[2]),
                            scalar2=None, op0=OP.mult)
                        nc.vector.tensor_tensor(out=dst, in0=ta[:], in1=dst,
                                                op=OP.add)
                        nc.vector.tensor_tensor(out=dst, in0=tb[:], in1=dst,
                                                op=OP.add)
                    else:
                        nc.scalar.copy(dst, pd[:])

                conv_blk(0)
                for j in range(1, nblk):
                    conv_blk(j)
                    dw_blk(j - 1)
                dw_blk(nblk - 1)

            if debug:
                nc.sync.dma_start(out=dbg_pre[:], in_=scratch[:])
                nc.sync.dma_start(out=dbg_q[:], in_=dw0[:])
                nc.sync.dma_start(out=dbg_k[:], in_=dw1[:])
            # ===== agent pooling (a = dw1[64:96], dw1[96:128], dw2[96:128])
            AGRP = ((dw1, 64, 0), (dw1, 96, 0), (dw2, 96, 1))
            for (abuf, w0, half) in AGRP:
                a3 = abuf[w0:w0 + 32, :].rearrange("p (a xi) -> p a xi", xi=16)
                s1 = as1[w0:w0 + 32, half * 1024:(half + 1) * 1024]
                nc.vector.reduce_sum(s1, a3, axis=AX.X)
                as3 = s1.rearrange("p (yb yi xb) -> p yb xb yi",
                                   yb=8, yi=16, xb=8)
                asum3 = asum[w0:w0 + 32, half * 64:(half + 1) * 64].rearrange(
                    "p (yb xb) -> p yb xb", yb=8)
                nc.vector.reduce_sum(asum3, as3, axis=AX.X)
            nc.sync.dma_start(out=agf[0:32, :], in_=asum[64:96, 0:64])
            nc.sync.dma_start(out=agf[32:64, :], in_=asum[96:128, 0:64])
            nc.sync.dma_start(out=agf[64:96, :], in_=asum[96:128, 64:128])

            # ============ vT build via PE transpose (into scratch) =======
            # per-chunk layout [one | v-ch 0:96 | one]: cols 0, 1:97, 97
            # 8 chunks (24 transposes of [32,128] -> [128,32] f16 in PSUM)
            # per ACT copy into the strided vT3 slots.
            vT3 = vT.rearrange("p (c w) -> p c w", w=98)
            nc.gpsimd.memset(vT3[:, :, 0], 1.0)
            nc.gpsimd.memset(vT3[:, :, 97], 1.0)
            for c0 in range(0, NCH, 5):
                cs = min(5, NCH - c0)
                pt = ppsum.tile([128, 512], F32, tag="pA", name="pt")
                for ci in range(cs):
                    ssl = slice((c0 + ci) * 128, (c0 + ci + 1) * 128)
                    nc.tensor.matmul(
                        pt[:, ci * 96:ci * 96 + 96],
                        dw2[0:96, ssl], idmat[:],
                        start=True, stop=True)
                nc.scalar.copy(vT3[:, c0:c0 + cs, 1:97], pt[:, 0:cs * 96])
            # scale by temp/256 (per-partition scalar), then place blocks by DMA
            nc.vector.tensor_scalar(out=agfs[:], in0=agf[:],
                                    scalar1=temp_rep[:], scalar2=1.0 / 256.0,
                                    op0=OP.mult, op1=OP.mult)
            for h in range(4):
                nc.sync.dma_start(
                    out=ag_full[h * 24:(h + 1) * 24, h * 64:(h + 1) * 64],
                    in_=agfs[h * 24:(h + 1) * 24, :])

            # ================= l2norm of q, k =========================
            for j in range(NB2):
                blk = slice(j * BLK2, (j + 1) * BLK2)
                sq0 = work1.tile([128, BLK2], F16, tag="sq0")
                sq1 = work1.tile([64, BLK2], F16, tag="sq1")
                sqk = work1.tile([32, BLK2], F16, tag="sqk")
                nc.gpsimd.tensor_tensor(out=sq0[:], in0=dw0[:, blk], in1=dw0[:, blk],
                                        op=OP.mult)
                nc.vector.tensor_tensor(out=sq1[:], in0=dw1[0:64, blk],
                                        in1=dw1[0:64, blk], op=OP.mult)
                nc.sync.dma_start(out=sqk[:], in_=sq0[96:128, :])
                pq = ppsum.tile([96, BLK2], F32, tag="pA")
                pk = ppsum.tile([96, BLK2], F32, tag="pB")
                for q in range(2):
                    sl = slice(q * 512, (q + 1) * 512)
                    nc.tensor.matmul(pq[:, sl], ones_q[:], sq0[0:96, sl],
                                     start=True, stop=True)
                    nc.tensor.matmul(pk[:, sl], ones_kA[:], sqk[:, sl],
                                     start=True, stop=False)
                    nc.tensor.matmul(pk[:, sl], ones_kB[:], sq1[:, sl],
                                     start=False, stop=True)
                rinv_q = work1.tile([96, BLK2], F16, tag="rinv_q")
                rinv_k = work1.tile([96, BLK2], F16, tag="rinv_k")
                nc.scalar.activation(rinv_q[:], pq[:], AF.Abs_reciprocal_sqrt)
                nc.scalar.activation(rinv_k[:], pk[:], AF.Abs_reciprocal_sqrt)
                rrk = work1.tile([128, BLK2], F16, tag="rrk")
                nc.sync.dma_start(out=rrk[96:128, :], in_=rinv_k[0:32, :])
                nc.sync.dma_start(out=rrk[0:64, :], in_=rinv_k[32:96, :])
                nc.vector.tensor_tensor(out=dw0[0:96, blk], in0=dw0[0:96, blk],
                                        in1=rinv_q[:], op=OP.mult)
                nc.gpsimd.tensor_tensor(out=dw0[96:128, blk], in0=dw0[96:128, blk],
                                        in1=rrk[96:128, :], op=OP.mult)
                nc.gpsimd.tensor_tensor(out=dw1[0:64, blk], in0=dw1[0:64, blk],
                                        in1=rrk[0:64, :], op=OP.mult)

            if debug:
                nc.sync.dma_start(out=dbg_qn[:], in_=dw0[:])
            # ====== pack k-hat contiguous at base 0 (into dw2 rows 0:96) ==
            # (v rows of dw2 already consumed by vT; a rows by pooling)
            nc.sync.dma_start(out=kfull[0:32, :], in_=dw0[96:128, :])
            nc.sync.dma_start(out=kfull[32:96, :], in_=dw1[0:64, :])

            # ================= k-side: L2T -> exp -> agent_v =============
            agv0 = ppsum.tile([128, 49], F32, tag="pB")
            agv1 = ppsum.tile([128, 49], F32, tag="pB")
            for c in range(NCH):
                ssl = slice(c * 128, (c + 1) * 128)
                l2 = ppsum.tile([128, 256], F32, tag="pA")
                nc.tensor.matmul(l2[:], kfull[:, ssl], ag_full[:],
                                 start=True, stop=True)
                e2t = work.tile([128, 256], F16, tag="e2t")
                nc.scalar.activation(e2t[:], l2[:], AF.Exp)
                nc.tensor.matmul(agv0[:], e2t[:, 0:128], vT3[:, c, 0:49],
                                 start=(c == 0), stop=(c == NCH - 1))
                nc.tensor.matmul(agv1[:], e2t[:, 128:256], vT3[:, c, 49:98],
                                 start=(c == 0), stop=(c == NCH - 1))
            # agv0: D2 col 0, channels cols 1:49. agv1: channels 0:48, D2 col 48.
            nc.vector.reciprocal_approx_fast(out=rq2a[:], in_=agv0[:, 0:1])
            nc.vector.reciprocal_approx_fast(out=rq2b[:], in_=agv1[:, 48:49])
            # block-diagonal: even head of pair -> rows 0:64 x cols 0:24,
            # odd head -> rows 64:128 x cols 24:48 (other entries stay zero)
            nc.vector.tensor_scalar(out=av_l0[0:64, 0:24], in0=agv0[0:64, 1:25],
                                    scalar1=rq2a[0:64, :], scalar2=None, op0=OP.mult)
            for w0 in (64, 96):
                nc.vector.tensor_scalar(out=av_l0[w0:w0 + 32, 24:48],
                                        in0=agv0[w0:w0 + 32, 25:49],
                                        scalar1=rq2a[w0:w0 + 32, :], scalar2=None,
                                        op0=OP.mult)
            nc.vector.tensor_scalar(out=av_l1[0:64, 0:24], in0=agv1[0:64, 0:24],
                                    scalar1=rq2b[0:64, :], scalar2=None, op0=OP.mult)
            for w0 in (64, 96):
                nc.vector.tensor_scalar(out=av_l1[w0:w0 + 32, 24:48],
                                        in0=agv1[w0:w0 + 32, 24:48],
                                        scalar1=rq2b[w0:w0 + 32, :], scalar2=None,
                                        op0=OP.mult)

            if debug:
                nc.sync.dma_start(out=dbg_ag[:], in_=ag_full[:])
                nc.sync.dma_start(out=dbg_av0[:], in_=av_l0[:])
                nc.sync.dma_start(out=dbg_av1[:], in_=av_l1[:])
                nc.sync.dma_start(out=dbg_vt[:], in_=vT[:, 0:98 * 4])
            # ================= q-side + division =========================
            # Both head-pairs per j-block: op_/od_ psum rows 0:48 (hp0) and
            # 64:112 (hp1); one recip + one STT over [128, BLK2] covers both.
            # x_attn rows 48:64 / 112:128 are junk, skipped at output DMA.
            for j in range(NB2):
                blk = slice(j * BLK2, (j + 1) * BLK2)
                e1s = []
                for hp in range(2):
                    ag_cols = ag_full[:, hp * 128:(hp + 1) * 128]
                    l1 = ppsum.tile([128, BLK2], F32, tag="pA", name="l1")
                    for q in range(2):
                        sl = slice(j * BLK2 + q * 512, j * BLK2 + (q + 1) * 512)
                        psl = slice(q * 512, (q + 1) * 512)
                        nc.tensor.matmul(l1[:, psl], ag_cols, dw0[0:96, sl],
                                         start=True, stop=True)
                    e1 = work.tile([128, BLK2], F16, tag=f"e1{hp}", name="e1")
                    nc.scalar.activation(e1[:], l1[:], AF.Exp)
                    e1s.append(e1)
                op_ = ppsum.tile([128, BLK2], F32, tag="pB", name="op_")
                od_ = ppsum.tile([128, BLK2], F32, tag="pB", name="od_")
                for hp in range(2):
                    rb = 64 * hp
                    av_l = av_l0 if hp == 0 else av_l1
                    for q in range(2):
                        psl = slice(q * 512, (q + 1) * 512)
                        nc.tensor.matmul(op_[rb:rb + 48, psl], av_l[:],
                                         e1s[hp][:, psl], start=True, stop=True)
                        nc.tensor.matmul(od_[rb:rb + 48, psl], dv_ones[:],
                                         e1s[hp][:, psl], start=True, stop=True)
                rqs = work1.tile([128, BLK2], F32, tag="rqs")
                nc.vector.reciprocal_approx_fast(out=rqs[:], in_=od_[:])
                nc.vector.scalar_tensor_tensor(
                    out=x_attn[:, blk], in0=op_[:], scalar=0.0,
                    in1=rqs[:], op0=OP.bypass, op1=OP.mult,
                    accum_out=mu_parts[:, j:j + 1])

            if debug:
                nc.sync.dma_start(out=dbg_xa[:], in_=x_attn[:])
            # ================= SimAM =====================================
            # all [128, *]: rows 48:64 / 112:128 are junk lanes, skipped at
            # the output DMAs; per-partition stats keep junk contained.
            nc.vector.reduce_sum(mub[:], mu_parts[:], axis=AX.X)
            nc.vector.tensor_scalar(out=mub[:], in0=mub[:],
                                    scalar1=-1.0 / N, scalar2=None, op0=OP.mult)
            for j in range(NB2):
                blk = slice(j * BLK2, (j + 1) * BLK2)
                nc.scalar.activation(d2[:, blk], x_attn[:, blk], AF.Square,
                                     bias=mub[:], scale=1.0,
                                     accum_out=d2_parts[:, j:j + 1])
            nc.vector.reduce_sum(sden[:], d2_parts[:], axis=AX.X)
            nc.vector.tensor_scalar(out=sden[:], in0=sden[:],
                                    scalar1=4.0 / (N - 1), scalar2=4.0 * E_LAMBDA,
                                    op0=OP.mult, op1=OP.add)
            nc.vector.reciprocal_approx_fast(out=s_ch[:], in_=sden[:])
            for j in range(NB2):
                blk = slice(j * BLK2, (j + 1) * BLK2)
                nc.scalar.activation(sig[:, blk], d2[:, blk], AF.Sigmoid,
                                     bias=half_s[:], scale=s_ch[:])
                ob = pout.tile([128, BLK2], F32, tag="ob")
                nc.vector.tensor_tensor(out=ob[:], in0=x_attn[:, blk],
                                        in1=sig[:, blk], op=OP.mult)
                nc.sync.dma_start(out=out_d[0:48, blk], in_=ob[0:48, :])
                nc.sync.dma_start(out=out_d[48:96, blk], in_=ob[64:112, :])

    nc.compile()
    return nc


_NC = None


def _get_nc():
    global _NC
    if _NC is None:
        _install_ntff_hook()
        _NC = build_nc()
    return _NC


def make_core_inputs(x, w_qkv, w_dw, temperature):
    """Host-side shard prep. Returns list of 8 input dicts."""
    x = np.asarray(x)
    w_qkv = np.asarray(w_qkv)
    w_dw = np.asarray(w_dw)
    temperature = np.asarray(temperature).reshape(8)
    in_maps = []
    for core in range(8):
        b, hg = core // 2, core % 2
        # slab0 = q + k[0:32]; slab1 = k[32:96] + a[0:64];
        # slab2 = v[0:96] + a[64:96]  (v at base 0 for PE transpose)
        rows = np.concatenate([
            np.arange(hg * 96, hg * 96 + 96),           # q
            192 + np.arange(hg * 96, hg * 96 + 96),     # k
            576 + np.arange(hg * 96, hg * 96 + 64),     # a[0:64]
            384 + np.arange(hg * 96, hg * 96 + 96),     # v
            576 + np.arange(hg * 96 + 64, hg * 96 + 96),  # a[64:96]
        ])
        W1 = w_qkv[rows, :, 0, 0]                        # [384, 192]
        W1T = np.ascontiguousarray(W1.T).astype(np.float16)
        wd9 = w_dw[rows, 0].reshape(384, 9).astype(np.float32)
        wdiag_h = np.zeros((128, NDIAG * 128), np.float16)
        wtap_h = np.zeros((128, 27), np.float32)
        for s in range(3):
            for t in range(9):
                wtap_h[:, s * 9 + t] = wd9[s * 128:(s + 1) * 128, t]
        for (s, dy, dx), idx in WDIAG_SLOT.items():
            t = (dy + 1) * 3 + (dx + 1)
            wdiag_h[np.arange(128), idx * 128 + np.arange(128)] = \
                wd9[s * 128:(s + 1) * 128, t].astype(np.float16)
        pat_h = np.zeros((128, 336), np.float16)
        pat_h[np.arange(96), 240 + np.arange(96)] = 1   # I96 for v transpose
        for h in range(4):
            pat_h[h * 24:(h + 1) * 24, h * 24:(h + 1) * 24] = 1    # ones_q
        # ones_kA (rows 0:32 of cols 96:192): k-ch 0:32 -> rep cols
        pat_h[0:24, 96 + 0:96 + 24] = 1
        pat_h[24:32, 96 + 24:96 + 48] = 1
        # ones_kB (rows 32:96 of cols 96:192): k-ch 32:96
        pat_h[32 + 0:32 + 16, 96 + 24:96 + 48] = 1
        pat_h[32 + 16:32 + 40, 96 + 48:96 + 72] = 1
        pat_h[32 + 40:32 + 64, 96 + 72:96 + 96] = 1
        # D1-rep ones: cols 192:216 (rows 0:64), cols 216:240 (rows 64:128)
        pat_h[0:64, 192:216] = 1
        pat_h[64:128, 216:240] = 1
        heads = np.arange(hg * 4, hg * 4 + 4)
        t4 = temperature[heads].astype(np.float32)
        in_maps.append({
            "xin": x[b].reshape(192, N).astype(np.float16),
            "w1a": W1T[0:96].copy(),
            "w1b": W1T[96:192].copy(),
            "wdiag": wdiag_h,
            "wtap": wtap_h,
            "tmp0": np.repeat(t4[0:2], 24).reshape(48, 1).copy(),
            "tmp1": np.repeat(t4[2:4], 24).reshape(48, 1).copy(),
            "pat": pat_h,
        })
    return in_maps


def _assemble(results):
    full = np.empty((B, C, H, W), np.float32)
    for core in range(8):
        b, hg = core // 2, core % 2
        full[b, hg * 96:(hg + 1) * 96] = results[core]["out"].reshape(96, H, W)
    return full


def kernel(x, w_qkv, w_dw, temperature):
    nc = _get_nc()
    in_maps = make_core_inputs(x, w_qkv, w_dw, temperature)
    res = run_bass_kernel_spmd(nc, in_maps, list(range(8)))
    return _assemble(res.results)


def kernel_profiled(x, w_qkv, w_dw, temperature):
    nc = _get_nc()
    in_maps = make_core_inputs(x, w_qkv, w_dw, temperature)
    res = run_bass_kernel_spmd(nc, in_maps, list(range(8)), trace=True)
    return _assemble(res.results), res.exec_time_ns



# revision 32
# speedup vs baseline: 3.3625x; 1.0273x over previous
"""MASA agent-attention kernel for Trainium2, 8-core SPMD.

Sharding: core = (batch b in 0..3) x (head-group hg in 0..1).
Each core computes conv1x1 + depthwise3x3 for its 4 heads' q/k/v/a
channels (384 of 768), the agent attention for those heads, and SimAM
over its 96 output channels. No cross-core communication.

Per-core channel order: [q(96), k(96), a(0:64), v(96), a(64:96)].
SBUF slabs of 128: s0 = q[0:96]+k[0:32], s1 = k[32:96]+a[0:64],
s2 = v[0:96]+a[64:96].  v at slab base 0 so the v-transpose is one
[96,128] PE matmul (vs identity) per 128-pixel chunk.

Engine-op partition windows must be 32-aligned and (base==0 or count<=32).
"""

import sys
import types
import numpy as np

import concourse.bacc as bacc
import concourse.bass as bass
import concourse.mybir as mybir
from concourse.tile import TileContext
from concourse.bass_utils import run_bass_kernel_spmd

F16 = mybir.dt.float16
F32 = mybir.dt.float32
AX = mybir.AxisListType
OP = mybir.AluOpType
AF = mybir.ActivationFunctionType

B, C, H, W = 4, 192, 128, 128
N = H * W              # 16384
M_AG = 64              # agent tokens
E_LAMBDA = 1e-4
RS = 130               # padded row stride for pre
PREFREE = RS * RS      # 16900

TAPS = [(dy, dx) for dy in (-1, 0, 1) for dx in (-1, 0, 1)]
# tap offset in pre: (1+dy)*RS + (1+dx); odd offsets (dx==0) are
# 4B-misaligned for fp16 2x mode -> always on PE. DVE gets only the
# dx=+1 column (aligned), as tensor_scalar products + tensor_tensor
# adds (packed modes); scalar_tensor_tensor is always 1x on DVE.
PE_TAPS = {
    0: TAPS,                                  # slab0 fully on PE
    1: [t for t in TAPS if t[1] <= 0],        # dx in {-1, 0}
    2: [t for t in TAPS if t[1] <= 0],
}
DVE_TAPS = {s: [t for t in TAPS if t not in PE_TAPS[s]] for s in range(3)}
WDIAG_SLOT = {}
for _s in range(3):
    for _t in PE_TAPS[_s]:
        WDIAG_SLOT[(_s, _t[0], _t[1])] = len(WDIAG_SLOT)
NDIAG = len(WDIAG_SLOT)

NB2 = 16               # block count for norm / attention / simam phases
BLK2 = 1024
NCH = 128              # s-chunks of 128 for k-side


def _install_ntff_hook():
    try:
        import antenv.axon_hooks  # noqa: F401
        return
    except ImportError:
        pass
    try:
        from trn_agent_boot.trn_boot import _ntff_profile_via_ctypes
        hook = _ntff_profile_via_ctypes('/opt/axon/libaxon_pjrt.so')
        mod = types.ModuleType("antenv.axon_hooks")
        mod.get_axon_ntff_profile_hook = lambda: hook
        mod.set_axon_ntff_profile_hook = lambda h: None
        sys.modules["antenv.axon_hooks"] = mod
    except Exception:
        pass


def build_nc(debug=False):
    nc = bacc.Bacc("TRN2", target_bir_lowering=False, debug=False, num_devices=8)

    # ---- DRAM I/O ----
    xin = nc.dram_tensor("xin", [192, N], F16, kind="ExternalInput").ap()
    w1a = nc.dram_tensor("w1a", [96, 384], F16, kind="ExternalInput").ap()
    w1b = nc.dram_tensor("w1b", [96, 384], F16, kind="ExternalInput").ap()
    wdiag = nc.dram_tensor("wdiag", [128, NDIAG * 128], F16, kind="ExternalInput").ap()
    wtap = nc.dram_tensor("wtap", [128, 27], F32, kind="ExternalInput").ap()
    tmp0 = nc.dram_tensor("tmp0", [48, 1], F32, kind="ExternalInput").ap()
    tmp1 = nc.dram_tensor("tmp1", [48, 1], F32, kind="ExternalInput").ap()
    pat = nc.dram_tensor("pat", [128, 336], F16, kind="ExternalInput").ap()
    out_d = nc.dram_tensor("out", [96, N], F32, kind="ExternalOutput").ap()
    if debug:
        dbg_pre = nc.dram_tensor("dbg_pre", [128, PREFREE], F16, kind="ExternalOutput").ap()
        dbg_q = nc.dram_tensor("dbg_q", [128, N], F16, kind="ExternalOutput").ap()
        dbg_k = nc.dram_tensor("dbg_k", [128, N], F16, kind="ExternalOutput").ap()
        dbg_qn = nc.dram_tensor("dbg_qn", [128, N], F16, kind="ExternalOutput").ap()
        dbg_ag = nc.dram_tensor("dbg_ag", [96, 256], F16, kind="ExternalOutput").ap()
        dbg_av0 = nc.dram_tensor("dbg_av0", [128, 48], F16, kind="ExternalOutput").ap()
        dbg_av1 = nc.dram_tensor("dbg_av1", [128, 48], F16, kind="ExternalOutput").ap()
        dbg_xa = nc.dram_tensor("dbg_xa", [96, N], F16, kind="ExternalOutput").ap()
        dbg_vt = nc.dram_tensor("dbg_vt", [128, 98 * 4], F16, kind="ExternalOutput").ap()
        dbg_avi = nc.dram_tensor("dbg_avi", [128, 48], F16, kind="ExternalOutput").ap()
        dbg_e1 = nc.dram_tensor("dbg_e1", [128, BLK2], F16, kind="ExternalOutput").ap()
        dbg_op = nc.dram_tensor("dbg_op", [128, BLK2], F32, kind="ExternalOutput").ap()
        dbg_rqs = nc.dram_tensor("dbg_rqs", [48, BLK2], F32, kind="ExternalOutput").ap()

    # ---- persistent SBUF ----
    scratch = nc.alloc_sbuf_tensor("scratch", [128, PREFREE], F16).ap()
    dw0 = nc.alloc_sbuf_tensor("dw0", [128, N], F16).ap()
    dw1 = nc.alloc_sbuf_tensor("dw1", [128, N], F16).ap()
    dw2 = nc.alloc_sbuf_tensor("dw2", [128, N], F16).ap()
    dws = [dw0, dw1, dw2]
    w1a_s = nc.alloc_sbuf_tensor("w1a_s", [96, 384], F16).ap()
    w1b_s = nc.alloc_sbuf_tensor("w1b_s", [96, 384], F16).ap()
    wdiag_s = nc.alloc_sbuf_tensor("wdiag_s", [128, NDIAG * 128], F16).ap()
    wtap_s = nc.alloc_sbuf_tensor("wtap_s", [128, 27], F32).ap()
    ones_q = nc.alloc_sbuf_tensor("ones_q", [96, 96], F16).ap()
    ones_kA = nc.alloc_sbuf_tensor("ones_kA", [32, 96], F16).ap()
    ones_kB = nc.alloc_sbuf_tensor("ones_kB", [64, 96], F16).ap()
    ag_full = nc.alloc_sbuf_tensor("ag_full", [96, 256], F16).ap()
    agf = nc.alloc_sbuf_tensor("agf", [96, M_AG], F32).ap()
    agfs = nc.alloc_sbuf_tensor("agfs", [96, M_AG], F16).ap()
    temp_rep = nc.alloc_sbuf_tensor("temp_rep", [96, 1], F32).ap()
    av_l0 = nc.alloc_sbuf_tensor("av_l0", [128, 48], F16).ap()
    av_l1 = nc.alloc_sbuf_tensor("av_l1", [128, 48], F16).ap()
    dv_ones = nc.alloc_sbuf_tensor("dv_ones", [128, 48], F16).ap()
    idmat = nc.alloc_sbuf_tensor("idmat", [96, 96], F16).ap()
    asum = nc.alloc_sbuf_tensor("asum", [128, 2 * M_AG], F32).ap()  # rows 64:128
    rq2a = nc.alloc_sbuf_tensor("rq2a", [128, 1], F32).ap()
    rq2b = nc.alloc_sbuf_tensor("rq2b", [128, 1], F32).ap()
    mu_parts = nc.alloc_sbuf_tensor("mu_parts", [128, NB2], F32).ap()
    x2_parts = nc.alloc_sbuf_tensor("x2_parts", [128, NB2], F32).ap()
    musum = nc.alloc_sbuf_tensor("musum", [128, 1], F32).ap()
    sx2 = nc.alloc_sbuf_tensor("sx2", [128, 1], F32).ap()
    sden = nc.alloc_sbuf_tensor("sden", [128, 1], F32).ap()
    s_ch = nc.alloc_sbuf_tensor("s_ch", [128, 1], F32).ap()
    sqs = nc.alloc_sbuf_tensor("sqs", [128, 1], F32).ap()
    biasb = nc.alloc_sbuf_tensor("biasb", [128, 1], F32).ap()
    half_s = nc.alloc_sbuf_tensor("half_s", [128, 1], F32).ap()

    # aliases (sequential reuse of big buffers)
    pre3 = scratch.rearrange("p (y x) -> p y x", x=RS)   # padded conv out
    vT = scratch[:, 0:NCH * 98]                          # after dwconv
    as1 = scratch[:, 12544:12544 + 4096].bitcast(F32)    # [128,2048] pool stage1
    x_attn = dw1[:, :]                                   # [128, N] f16 (phase D)
    kfull = dw2[0:96, :]                                 # k-hat packed (phase B)

    with TileContext(nc) as tc:
        with (
            tc.tile_pool(name="xio", bufs=4) as xio,
            tc.tile_pool(name="pout", bufs=2) as pout,
            tc.tile_pool(name="work", bufs=2) as work,
            tc.tile_pool(name="work1", bufs=1) as work1,
            tc.tile_pool(name="ppsum", bufs=2, space="PSUM") as ppsum,
        ):
            # ================= init =================
            nc.sync.dma_start(out=w1a_s[:], in_=w1a[:])
            nc.sync.dma_start(out=w1b_s[:], in_=w1b[:])
            nc.sync.dma_start(out=wdiag_s[:], in_=wdiag[:])
            nc.sync.dma_start(out=wtap_s[:], in_=wtap[:])
            # static patterns
            nc.sync.dma_start(out=ones_q[:], in_=pat[0:96, 0:96])
            nc.sync.dma_start(out=ones_kA[:], in_=pat[0:32, 96:192])
            nc.sync.dma_start(out=ones_kB[:], in_=pat[32:96, 96:192])
            nc.gpsimd.memset(av_l0[:], 0.0)
            nc.gpsimd.memset(av_l1[:], 0.0)
            # D1-rep ones lhsT: col j<24 -> even head (rows 0:64),
            # j>=24 -> odd head (rows 64:128)
            nc.sync.dma_start(out=dv_ones[:, 0:24], in_=pat[:, 192:216])
            nc.sync.dma_start(out=dv_ones[:, 24:48], in_=pat[:, 216:240])
            nc.sync.dma_start(out=idmat[:], in_=pat[0:96, 240:336])
            nc.gpsimd.memset(ag_full[:], 0.0)
            nc.sync.dma_start(out=temp_rep[0:48, :], in_=tmp0[:])
            nc.sync.dma_start(out=temp_rep[48:96, :], in_=tmp1[:])
            nc.gpsimd.memset(half_s[:], 0.5)
            # pre borders (rows 0 and 129, cols 0 and 129)
            nc.gpsimd.memset(pre3[:, 0, :], 0.0)
            nc.gpsimd.memset(pre3[:, 129, :], 0.0)
            nc.gpsimd.memset(pre3[:, :, 0], 0.0)
            nc.gpsimd.memset(pre3[:, :, 129], 0.0)

            if debug:
                nc.sync.dma_start(out=dbg_avi[:], in_=dv_ones[:])
            # ================= sweep1: conv1x1 + dwconv ====
            for s in range(3):
                wa = w1a_s[:, s * 128:(s + 1) * 128]
                wb = w1b_s[:, s * 128:(s + 1) * 128]
                nblk = N // 1024  # 16 blocks of 1024 (8 y-rows)

                def conv_blk(j, s=s, wa=wa, wb=wb):
                    x0 = xio.tile([96, 1024], F16, tag="x")
                    x1 = xio.tile([96, 1024], F16, tag="x")
                    nc.sync.dma_start(out=x0[:], in_=xin[0:96, j * 1024:(j + 1) * 1024])
                    nc.sync.dma_start(out=x1[:], in_=xin[96:192, j * 1024:(j + 1) * 1024])
                    ps = ppsum.tile([128, 1024], F32, tag="pA")
                    for q in range(2):
                        sl = slice(q * 512, (q + 1) * 512)
                        nc.tensor.matmul(ps[:, sl], wa, x0[:, sl], start=True, stop=False)
                        nc.tensor.matmul(ps[:, sl], wb, x1[:, sl], start=False, stop=True)
                    nc.scalar.copy(pre3[:, 1 + 8 * j: 9 + 8 * j, 1:129], ps[:])

                def dw_blk(j, s=s):
                    dst = dws[s][:, j * 1024:(j + 1) * 1024]
                    pe_t = PE_TAPS[s]
                    dv_t = DVE_TAPS[s]
                    pd = None
                    if pe_t:
                        pd = ppsum.tile([128, 1024], F32, tag="pB")
                        for q in range(2):
                            for ti, (dy, dx) in enumerate(pe_t):
                                dg = wdiag_s[:, WDIAG_SLOT[(s, dy, dx)] * 128:
                                             (WDIAG_SLOT[(s, dy, dx)] + 1) * 128]
                                rv = pre3[:, 1 + dy + 8 * j + 4 * q: 5 + dy + 8 * j + 4 * q,
                                          1 + dx: 129 + dx]
                                nc.tensor.matmul(pd[:, q * 512:(q + 1) * 512], dg, rv,
                                                 start=(ti == 0), stop=(ti == len(pe_t) - 1))
                    if dv_t:
                        # 3 aligned taps: 1 STT (merges PE psum, 1x) +
                        # 2 TS products (4x) + 2 TT adds (2x)
                        def win(dy, dx):
                            return pre3[:, 1 + dy + 8 * j: 9 + dy + 8 * j,
                                        1 + dx: 129 + dx]

                        def wsc(dy, dx):
                            ti = s * 9 + TAPS.index((dy, dx))
                            return wtap_s[:, ti:ti + 1]

                        ta = work.tile([128, 1024], F16, tag="dta")
                        nc.vector.scalar_tensor_tensor(
                            out=ta[:], in0=win(*dv_t[0]), scalar=wsc(*dv_t[0]),
                            in1=pd[:], op0=OP.mult, op1=OP.add)
                        tb = work.tile([128, 1024], F16, tag="dtb")
                        nc.vector.tensor_scalar(
                            out=tb[:], in0=win(*dv_t[1]), scalar1=wsc(*dv_t[1]),
                            scalar2=None, op0=OP.mult)
                        nc.vector.tensor_scalar(
                            out=dst, in0=win(*dv_t[2]), scalar1=wsc(*dv_t[2]),
                            scalar2=None, op0=OP.mult)
                        nc.vector.tensor_tensor(out=dst, in0=ta[:], in1=dst,
                                                op=OP.add)
                        nc.vector.tensor_tensor(out=dst, in0=tb[:], in1=dst,
                                                op=OP.add)
                    else:
                        nc.scalar.copy(dst, pd[:])

                conv_blk(0)
                for j in range(1, nblk):
                    conv_blk(j)
                    dw_blk(j - 1)
                dw_blk(nblk - 1)

            if debug:
                nc.sync.dma_start(out=dbg_pre[:], in_=scratch[:])
                nc.sync.dma_start(out=dbg_q[:], in_=dw0[:])
                nc.sync.dma_start(out=dbg_k[:], in_=dw1[:])
            # ===== agent pooling (a = dw1[64:96], dw1[96:128], dw2[96:128])
            AGRP = ((dw1, 64, 0), (dw1, 96, 0), (dw2, 96, 1))
            for (abuf, w0, half) in AGRP:
                a3 = abuf[w0:w0 + 32, :].rearrange("p (a xi) -> p a xi", xi=16)
                s1 = as1[w0:w0 + 32, half * 1024:(half + 1) * 1024]
                nc.vector.reduce_sum(s1, a3, axis=AX.X)
                as3 = s1.rearrange("p (yb yi xb) -> p yb xb yi",
                                   yb=8, yi=16, xb=8)
                asum3 = asum[w0:w0 + 32, half * 64:(half + 1) * 64].rearrange(
                    "p (yb xb) -> p yb xb", yb=8)
                nc.vector.reduce_sum(asum3, as3, axis=AX.X)
            nc.sync.dma_start(out=agf[0:32, :], in_=asum[64:96, 0:64])
            nc.sync.dma_start(out=agf[32:64, :], in_=asum[96:128, 0:64])
            nc.sync.dma_start(out=agf[64:96, :], in_=asum[96:128, 64:128])

            # ============ vT build via PE transpose (into scratch) =======
            # per-chunk layout [one | v-ch 0:96 | one]: cols 0, 1:97, 97
            # 8 chunks (24 transposes of [32,128] -> [128,32] f16 in PSUM)
            # per ACT copy into the strided vT3 slots.
            vT3 = vT.rearrange("p (c w) -> p c w", w=98)
            nc.gpsimd.memset(vT3[:, :, 0], 1.0)
            nc.gpsimd.memset(vT3[:, :, 97], 1.0)
            for c0 in range(0, NCH, 5):
                cs = min(5, NCH - c0)
                pt = ppsum.tile([128, 512], F32, tag="pA", name="pt")
                for ci in range(cs):
                    ssl = slice((c0 + ci) * 128, (c0 + ci + 1) * 128)
                    nc.tensor.matmul(
                        pt[:, ci * 96:ci * 96 + 96],
                        dw2[0:96, ssl], idmat[:],
                        start=True, stop=True)
                nc.scalar.copy(vT3[:, c0:c0 + cs, 1:97], pt[:, 0:cs * 96])
            # scale by temp/256 (per-partition scalar), then place blocks by DMA
            nc.vector.tensor_scalar(out=agfs[:], in0=agf[:],
                                    scalar1=temp_rep[:], scalar2=1.0 / 256.0,
                                    op0=OP.mult, op1=OP.mult)
            for h in range(4):
                nc.sync.dma_start(
                    out=ag_full[h * 24:(h + 1) * 24, h * 64:(h + 1) * 64],
                    in_=agfs[h * 24:(h + 1) * 24, :])

            # ================= l2norm of q, k =========================
            for j in range(NB2):
                blk = slice(j * BLK2, (j + 1) * BLK2)
                sq0 = work1.tile([128, BLK2], F16, tag="sq0")
                sq1 = work1.tile([64, BLK2], F16, tag="sq1")
                sqk = work1.tile([32, BLK2], F16, tag="sqk")
                nc.gpsimd.tensor_tensor(out=sq0[:], in0=dw0[:, blk], in1=dw0[:, blk],
                                        op=OP.mult)
                nc.vector.tensor_tensor(out=sq1[:], in0=dw1[0:64, blk],
                                        in1=dw1[0:64, blk], op=OP.mult)
                nc.sync.dma_start(out=sqk[:], in_=sq0[96:128, :])
                pq = ppsum.tile([96, BLK2], F32, tag="pA")
                pk = ppsum.tile([96, BLK2], F32, tag="pB")
                for q in range(2):
                    sl = slice(q * 512, (q + 1) * 512)
                    nc.tensor.matmul(pq[:, sl], ones_q[:], sq0[0:96, sl],
                                     start=True, stop=True)
                    nc.tensor.matmul(pk[:, sl], ones_kA[:], sqk[:, sl],
                                     start=True, stop=False)
                    nc.tensor.matmul(pk[:, sl], ones_kB[:], sq1[:, sl],
                                     start=False, stop=True)
                rinv_q = work1.tile([96, BLK2], F16, tag="rinv_q")
                rinv_k = work1.tile([96, BLK2], F16, tag="rinv_k")
                nc.scalar.activation(rinv_q[:], pq[:], AF.Abs_reciprocal_sqrt)
                nc.scalar.activation(rinv_k[:], pk[:], AF.Abs_reciprocal_sqrt)
                rrk = work1.tile([128, BLK2], F16, tag="rrk")
                nc.sync.dma_start(out=rrk[96:128, :], in_=rinv_k[0:32, :])
                nc.sync.dma_start(out=rrk[0:64, :], in_=rinv_k[32:96, :])
                nc.vector.tensor_tensor(out=dw0[0:96, blk], in0=dw0[0:96, blk],
                                        in1=rinv_q[:], op=OP.mult)
                nc.vector.tensor_tensor(out=dw0[96:128, blk], in0=dw0[96:128, blk],
                                        in1=rrk[96:128, :], op=OP.mult)
                nc.gpsimd.tensor_tensor(out=dw1[0:64, blk], in0=dw1[0:64, blk],
                                        in1=rrk[0:64, :], op=OP.mult)
                # pack k-hat per block so the k-side can start early
                # (v rows of dw2 consumed by vT; a rows by pooling)
                nc.sync.dma_start(out=kfull[0:32, blk], in_=dw0[96:128, blk])
                nc.sync.dma_start(out=kfull[32:96, blk], in_=dw1[0:64, blk])

            if debug:
                nc.sync.dma_start(out=dbg_qn[:], in_=dw0[:])

            # ================= k-side: L2T -> exp -> agent_v =============
            agv0 = ppsum.tile([128, 49], F32, tag="pB")
            agv1 = ppsum.tile([128, 49], F32, tag="pB")
            for c in range(NCH):
                ssl = slice(c * 128, (c + 1) * 128)
                l2 = ppsum.tile([128, 256], F32, tag="pA")
                nc.tensor.matmul(l2[:], kfull[:, ssl], ag_full[:],
                                 start=True, stop=True)
                e2t = work.tile([128, 256], F16, tag="e2t")
                nc.scalar.activation(e2t[:], l2[:], AF.Exp)
                nc.tensor.matmul(agv0[:], e2t[:, 0:128], vT3[:, c, 0:49],
                                 start=(c == 0), stop=(c == NCH - 1))
                nc.tensor.matmul(agv1[:], e2t[:, 128:256], vT3[:, c, 49:98],
                                 start=(c == 0), stop=(c == NCH - 1))
            # agv0: D2 col 0, channels cols 1:49. agv1: channels 0:48, D2 col 48.
            nc.vector.reciprocal_approx_fast(out=rq2a[:], in_=agv0[:, 0:1])
            nc.vector.reciprocal_approx_fast(out=rq2b[:], in_=agv1[:, 48:49])
            # block-diagonal: even head of pair -> rows 0:64 x cols 0:24,
            # odd head -> rows 64:128 x cols 24:48 (other entries stay zero)
            nc.vector.tensor_scalar(out=av_l0[0:64, 0:24], in0=agv0[0:64, 1:25],
                                    scalar1=rq2a[0:64, :], scalar2=None, op0=OP.mult)
            for w0 in (64, 96):
                nc.vector.tensor_scalar(out=av_l0[w0:w0 + 32, 24:48],
                                        in0=agv0[w0:w0 + 32, 25:49],
                                        scalar1=rq2a[w0:w0 + 32, :], scalar2=None,
                                        op0=OP.mult)
            nc.vector.tensor_scalar(out=av_l1[0:64, 0:24], in0=agv1[0:64, 0:24],
                                    scalar1=rq2b[0:64, :], scalar2=None, op0=OP.mult)
            for w0 in (64, 96):
                nc.vector.tensor_scalar(out=av_l1[w0:w0 + 32, 24:48],
                                        in0=agv1[w0:w0 + 32, 24:48],
                                        scalar1=rq2b[w0:w0 + 32, :], scalar2=None,
                                        op0=OP.mult)

            if debug:
                nc.sync.dma_start(out=dbg_ag[:], in_=ag_full[:])
                nc.sync.dma_start(out=dbg_av0[:], in_=av_l0[:])
                nc.sync.dma_start(out=dbg_av1[:], in_=av_l1[:])
                nc.sync.dma_start(out=dbg_vt[:], in_=vT[:, 0:98 * 4])
            # ================= q-side + division =========================
            # Both head-pairs per j-block: op_/od_ psum rows 0:48 (hp0) and
            # 64:112 (hp1); one recip + one STT over [128, BLK2] covers both.
            # x_attn rows 48:64 / 112:128 are junk, skipped at output DMA.
            for j in range(NB2):
                blk = slice(j * BLK2, (j + 1) * BLK2)
                e1s = []
                for hp in range(2):
                    ag_cols = ag_full[:, hp * 128:(hp + 1) * 128]
                    l1 = ppsum.tile([128, BLK2], F32, tag="pA", name="l1")
                    for q in range(2):
                        sl = slice(j * BLK2 + q * 512, j * BLK2 + (q + 1) * 512)
                        psl = slice(q * 512, (q + 1) * 512)
                        nc.tensor.matmul(l1[:, psl], ag_cols, dw0[0:96, sl],
                                         start=True, stop=True)
                    e1 = work.tile([128, BLK2], F16, tag=f"e1{hp}", name="e1")
                    nc.scalar.activation(e1[:], l1[:], AF.Exp)
                    e1s.append(e1)
                op_ = ppsum.tile([128, BLK2], F32, tag="pB", name="op_")
                od_ = ppsum.tile([128, BLK2], F32, tag="pB", name="od_")
                for hp in range(2):
                    rb = 64 * hp
                    av_l = av_l0 if hp == 0 else av_l1
                    for q in range(2):
                        psl = slice(q * 512, (q + 1) * 512)
                        nc.tensor.matmul(op_[rb:rb + 48, psl], av_l[:],
                                         e1s[hp][:, psl], start=True, stop=True)
                        nc.tensor.matmul(od_[rb:rb + 48, psl], dv_ones[:],
                                         e1s[hp][:, psl], start=True, stop=True)
                rqs = work1.tile([128, BLK2], F32, tag="rqs")
                nc.vector.reciprocal_approx_fast(out=rqs[:], in_=od_[:])
                nc.vector.scalar_tensor_tensor(
                    out=x_attn[:, blk], in0=op_[:], scalar=0.0,
                    in1=rqs[:], op0=OP.bypass, op1=OP.mult,
                    accum_out=mu_parts[:, j:j + 1])
                x2t = work.tile([128, BLK2], F16, tag="x2t")
                nc.scalar.activation(x2t[:], x_attn[:, blk], AF.Square,
                                     accum_out=x2_parts[:, j:j + 1])

            if debug:
                nc.sync.dma_start(out=dbg_xa[:], in_=x_attn[:])
            # ================= SimAM =====================================
            # all [128, *]: rows 48:64 / 112:128 are junk lanes, skipped at
            # the output DMAs; per-partition stats keep junk contained.
            # sum(d2) = sum(x^2) - N*mu^2 (both accumulated in the q-side),
            # and s*(x-mu)^2 = (sqrt(s)*x - sqrt(s)*mu)^2 folds into one
            # Square activation, so no separate d2 pass over N is needed.
            nc.vector.reduce_sum(musum[:], mu_parts[:], axis=AX.X)
            nc.vector.reduce_sum(sx2[:], x2_parts[:], axis=AX.X)
            mu2 = work1.tile([128, 1], F32, tag="mu2")
            nc.vector.tensor_tensor(out=mu2[:], in0=musum[:], in1=musum[:],
                                    op=OP.mult)
            nc.vector.scalar_tensor_tensor(
                out=sden[:], in0=mu2[:], scalar=-1.0 / N, in1=sx2[:],
                op0=OP.mult, op1=OP.add)
            nc.vector.tensor_scalar(out=sden[:], in0=sden[:],
                                    scalar1=4.0 / (N - 1), scalar2=4.0 * E_LAMBDA,
                                    op0=OP.mult, op1=OP.add)
            nc.vector.reciprocal_approx_fast(out=s_ch[:], in_=sden[:])
            nc.scalar.activation(sqs[:], s_ch[:], AF.Sqrt)
            nc.vector.tensor_scalar(out=biasb[:], in0=musum[:], scalar1=sqs[:],
                                    scalar2=-1.0 / N, op0=OP.mult, op1=OP.mult)
            for j in range(NB2):
                blk = slice(j * BLK2, (j + 1) * BLK2)
                d2t = work.tile([128, BLK2], F16, tag="d2t")
                nc.scalar.activation(d2t[:], x_attn[:, blk], AF.Square,
                                     bias=biasb[:], scale=sqs[:])
                sig_t = work.tile([128, BLK2], F16, tag="sig_t")
                nc.scalar.activation(sig_t[:], d2t[:], AF.Sigmoid,
                                     bias=half_s[:])
                ob = pout.tile([128, BLK2], F32, tag="ob")
                nc.vector.tensor_tensor(out=ob[:], in0=x_attn[:, blk],
                                        in1=sig_t[:], op=OP.mult)
                nc.sync.dma_start(out=out_d[0:48, blk], in_=ob[0:48, :])
                nc.sync.dma_start(out=out_d[48:96, blk], in_=ob[64:112, :])

    nc.compile()
    return nc


_NC = None


def _get_nc():
    global _NC
    if _NC is None:
        _install_ntff_hook()
        _NC = build_nc()
    return _NC


def make_core_inputs(x, w_qkv, w_dw, temperature):
    """Host-side shard prep. Returns list of 8 input dicts."""
    x = np.asarray(x)
    w_qkv = np.asarray(w_qkv)
    w_dw = np.asarray(w_dw)
    temperature = np.asarray(temperature).reshape(8)
    in_maps = []
    for core in range(8):
        b, hg = core // 2, core % 2
        # slab0 = q + k[0:32]; slab1 = k[32:96] + a[0:64];
        # slab2 = v[0:96] + a[64:96]  (v at base 0 for PE transpose)
        rows = np.concatenate([
            np.arange(hg * 96, hg * 96 + 96),           # q
            192 + np.arange(hg * 96, hg * 96 + 96),     # k
            576 + np.arange(hg * 96, hg * 96 + 64),     # a[0:64]
            384 + np.arange(hg * 96, hg * 96 + 96),     # v
            576 + np.arange(hg * 96 + 64, hg * 96 + 96),  # a[64:96]
        ])
        W1 = w_qkv[rows, :, 0, 0]                        # [384, 192]
        W1T = np.ascontiguousarray(W1.T).astype(np.float16)
        wd9 = w_dw[rows, 0].reshape(384, 9).astype(np.float32)
        wdiag_h = np.zeros((128, NDIAG * 128), np.float16)
        wtap_h = np.zeros((128, 27), np.float32)
        for s in range(3):
            for t in range(9):
                wtap_h[:, s * 9 + t] = wd9[s * 128:(s + 1) * 128, t]
        for (s, dy, dx), idx in WDIAG_SLOT.items():
            t = (dy + 1) * 3 + (dx + 1)
            wdiag_h[np.arange(128), idx * 128 + np.arange(128)] = \
                wd9[s * 128:(s + 1) * 128, t].astype(np.float16)
        pat_h = np.zeros((128, 336), np.float16)
        pat_h[np.arange(96), 240 + np.arange(96)] = 1   # I96 for v transpose
        for h in range(4):
            pat_h[h * 24:(h + 1) * 24, h * 24:(h + 1) * 24] = 1    # ones_q
        # ones_kA (rows 0:32 of cols 96:192): k-ch 0:32 -> rep cols
        pat_h[0:24, 96 + 0:96 + 24] = 1
        pat_h[24:32, 96 + 24:96 + 48] = 1
        # ones_kB (rows 32:96 of cols 96:192): k-ch 32:96
        pat_h[32 + 0:32 + 16, 96 + 24:96 + 48] = 1
        pat_h[32 + 16:32 + 40, 96 + 48:96 + 72] = 1
        pat_h[32 + 40:32 + 64, 96 + 72:96 + 96] = 1
        # D1-rep ones: cols 192:216 (rows 0:64), cols 216:240 (rows 64:128)
        pat_h[0:64, 192:216] = 1
        pat_h[64:128, 216:240] = 1
        heads = np.arange(hg * 4, hg * 4 + 4)
        t4 = temperature[heads].astype(np.float32)
        in_maps.append({
            "xin": x[b].reshape(192, N).astype(np.float16),
            "w1a": W1T[0:96].copy(),
            "w1b": W1T[96:192].copy(),
            "wdiag": wdiag_h,
            "wtap": wtap_h,
            "tmp0": np.repeat(t4[0:2], 24).reshape(48, 1).copy(),
            "tmp1": np.repeat(t4[2:4], 24).reshape(48, 1).copy(),
            "pat": pat_h,
        })
    return in_maps


def _assemble(results):
    full = np.empty((B, C, H, W), np.float32)
    for core in range(8):
        b, hg = core // 2, core % 2
        full[b, hg * 96:(hg + 1) * 96] = results[core]["out"].reshape(96, H, W)
    return full


def kernel(x, w_qkv, w_dw, temperature):
    nc = _get_nc()
    in_maps = make_core_inputs(x, w_qkv, w_dw, temperature)
    res = run_bass_kernel_spmd(nc, in_maps, list(range(8)))
    return _assemble(res.results)


def kernel_profiled(x, w_qkv, w_dw, temperature):
    nc = _get_nc()
    in_maps = make_core_inputs(x, w_qkv, w_dw, temperature)
    res = run_bass_kernel_spmd(nc, in_maps, list(range(8)), trace=True)
    return _assemble(res.results), res.exec_time_ns

